# revision 1
# baseline (speedup 1.0000x reference)
"""Trainium2 Bass kernel for nn_DHT_Layer (conv1x1+BN+ReLU -> Deep Hough
Transform -> two 3x3 conv+BN+ReLU layers).

Sharding: data-parallel over batch. 8 images / 8 cores -> one image per core,
no collectives; full inputs in, full output out. Inside each core:
  conv1   : 1x1 conv as 2 K-chunk matmuls (bf16); BN+ReLU epilogues
            alternate between ACT and DVE and scatter straight into a
            pixel-blocked h1 layout (so the PE transposes read contiguous
            chunks, as walrus requires).
  DHT     : out[c,a,r] = sum_p h[c,p] * (idx[a,p]==r) as one-hot matmuls.
            Pixels are chunked as 10x10 *2D blocks* (100 chunks of 100
            pixels): the rho window of a block is (10|cos|+10|sin|)/irho+1
            ~ 8-10 bins for every angle, vs ~32 for 128 contiguous pixels.
            One angle per accumulator slot (gl=1), 4 angles per PSUM bank
            (matching the conv2 row band), per-chunk window slicing: the
            matmul streams only win(a,chunk) columns -> ~91k total PE
            columns for the whole DHT (the PE cost model charges
            out-columns only; stationary loads are free).
            The one-hot is built on DVE as is_equal over a (j,k) layout:
            out[p, j*100+k] = (J2[p, j*100+k] == idxrel[p, a*100+k]) where
            J2 is a materialized j-iota table. All operands are 2-byte with
            packed last dims -> DVE 2x_1p mode (0.52 ns/col).
  conv2/3 : 3x3 convs as 9 shifted matmuls over a zero-padded [c, 102*102]
            layout, BN+ReLU folded into the epilogue; interleaved into the
            DHT band loop (conv2 lags 2 bands, conv3 lags 4) so PE never
            waits on an ACT drain it just requested.

Cost-model device time: ~138 us/core (PE ~92% busy; PE streams ~300k
matmul columns: conv2/3 180k, DHT 91k, conv1 20k, transposes 13k).

The local walrus build only supports ONE sync-wait per instruction, so a
post-pass splits multi-wait instructions into single-wait NoOp carriers.
"""

import functools
import math

import ml_dtypes
import numpy as np

N = 8          # batch / cores
CIN = 256
CMID = 128
H = W = 100
HW = H * W
A = 100        # angles
R = 100        # rho bins
BH, BW = 10, 10  # pixel block shape
YBLK = 10
XBLK = 10
NBLK = YBLK * XBLK             # 100 chunks
BPFULL = BH * BW               # 100 pixels per chunk (contraction dim)
BPTAIL = BH * BW               # uniform tiling: tail row is full height
WMAX = 10      # max rho window of any (angle, block)
SROWS = 5      # conv1 slice height (rows per psum chunk)
BAND = 4       # angles per PSUM bank / conv2 row band
NBAND = A // BAND
PADW = W + 2   # 102 padded spatial for 3x3 convs
BN_EPS = 1e-5
BF16 = ml_dtypes.bfloat16


# ----------------------------------------------------------------------------
# host-side precomputation (shapes are fixed -> cache)
# ----------------------------------------------------------------------------

def _hough_idx():
    irho = int(math.sqrt(H * H + W * W) + 1) / float(R)
    theta = np.arange(A) * (math.pi / A)
    tab_cos = np.cos(theta) / irho
    tab_sin = np.sin(theta) / irho
    yy, xx = np.meshgrid(np.arange(H) - H // 2, np.arange(W) - W // 2,
                         indexing='ij')
    r = np.round(xx[None, :, :] * tab_cos[:, None, None]
                 + yy[None, :, :] * tab_sin[:, None, None])
    return np.clip(r + R // 2, 0, R - 1).astype(np.int32)  # [A, H, W]


def _chunk_bp(b):
    return BPTAIL if b >= (YBLK - 1) * XBLK else BPFULL


def _tail_rows():
    return H - (YBLK - 1) * BH


@functools.lru_cache(maxsize=1)
def _dht_tables():
    idx = _hough_idx()
    # chunk b = gy*10+gx ; pixel partition p = dy*10+dx
    lo = np.zeros((A, NBLK), np.int64)
    win = np.zeros((A, NBLK), np.int64)
    idxrel = np.full((128, A * NBLK), 1000.0, np.float32)
    for gy in range(YBLK):
        hh = BH if gy < YBLK - 1 else _tail_rows()
        for gx in range(XBLK):
            b = gy * XBLK + gx
            sub = idx[:, gy * BH:gy * BH + hh,
                      gx * BW:(gx + 1) * BW].reshape(A, hh * BW)
            lo[:, b] = sub.min(axis=1)
            win[:, b] = sub.max(axis=1) - lo[:, b] + 1
            idxrel[:hh * BW, b::NBLK] = (sub - lo[:, b:b + 1]).T
    win_a = win.max(axis=1)                  # [A], <= WMAX
    assert int(win_a.max()) <= WMAX
    # J2 [128, WMAX*NBLK]: col j*NBLK+k -> j (identical rows)
    j2 = np.tile(np.repeat(np.arange(WMAX, dtype=np.float32), NBLK)[None, :],
                 (128, 1))
    return dict(lo=lo, win=win, win_a=win_a.astype(np.int64),
                idxrel=np.ascontiguousarray(idxrel.astype(BF16)),
                j2=np.ascontiguousarray(j2.astype(BF16)))


def _prep_weights(w1, b1, g1, be1, m1, v1, w2, b2, g2, be2, m2, v2,
                  w3, b3, g3, be3, m3, v3):
    s1 = g1 / np.sqrt(v1 + BN_EPS)
    s2 = g2 / np.sqrt(v2 + BN_EPS)
    s3 = g3 / np.sqrt(v3 + BN_EPS)
    # conv1: y[co] = sum_ci w1[co,ci]*x[ci]; fold BN scale into co rows.
    # DRAM layout [ci%128, half*128+co] -> one contiguous-row DMA.
    w1f = (w1[:, :, 0, 0] * s1[:, None]).T            # [ci=256, co=128]
    w1p = np.ascontiguousarray(
        w1f.reshape(2, 128, 128).transpose(1, 0, 2).reshape(128, 256)
        .astype(BF16))
    bias1 = ((b1 - m1) * s1 + be1).astype(np.float32).reshape(128, 1)
    # conv2/3: DRAM layout [ci, tap*128+co] -> one contiguous-row DMA.
    w2f = (w2 * s2[:, None, None, None]).transpose(2, 3, 1, 0)  # [ky,kx,ci,co]
    w2p = np.ascontiguousarray(
        w2f.reshape(9, 128, 128).transpose(1, 0, 2).reshape(128, 9 * 128)
        .astype(BF16))
    bias2 = ((b2 - m2) * s2 + be2).astype(np.float32).reshape(128, 1)
    w3f = (w3 * s3[:, None, None, None]).transpose(2, 3, 1, 0)
    w3p = np.ascontiguousarray(
        w3f.reshape(9, 128, 128).transpose(1, 0, 2).reshape(128, 9 * 128)
        .astype(BF16))
    bias3 = ((b3 - m3) * s3 + be3).astype(np.float32).reshape(128, 1)
    ident = np.eye(128, dtype=BF16)
    return w1p, bias1, w2p, bias2, w3p, bias3, ident


# ----------------------------------------------------------------------------
# walrus workaround: split multi-wait instructions (this build supports only
# one sync-wait per instruction)
# ----------------------------------------------------------------------------

def _split_multi_waits(nc, mybir, max_waits=1):
    cnt = 0
    for f in nc.m.functions:
        for bb in f.blocks:
            insts = list(bb.instructions)
            new = []
            changed = False
            for inst in insts:
                si = inst.sync_info
                if si is not None:
                    ow = list(si.on_wait)
                    if len(ow) > max_waits:
                        changed = True
                        head = ow[:-max_waits]
                        for i in range(0, len(head), max_waits):
                            nop = mybir.InstNoOp(name=f'waitsplit_{cnt}',
                                                 ins=[], outs=[])
                            cnt += 1
                            nop.engine = inst.engine
                            nop.sync_info = mybir.SyncInfo(
                                on_wait=head[i:i + max_waits], on_update=[])
                            new.append(nop)
                        si.on_wait = ow[-max_waits:]
                new.append(inst)
            if changed:
                bb.instructions = new
    return cnt


# ----------------------------------------------------------------------------
# bass program
# ----------------------------------------------------------------------------

_PROGRAM_CACHE = {}


def _build_program(split_waits=True):
    key = ('nc', split_waits)
    if key in _PROGRAM_CACHE:
        return _PROGRAM_CACHE[key]
    import concourse.bass as bass
    import concourse.mybir as mybir
    import concourse.tile as tile
    from contextlib import ExitStack

    T = _dht_tables()
    LO = T['lo']
    WIN = T['win']
    WIN_A = T['win_a']

    f32 = mybir.dt.float32
    bf16 = mybir.dt.bfloat16
    RELU = mybir.ActivationFunctionType.Relu
    COPY = mybir.ActivationFunctionType.Copy

    nc = bass.Bass('TRN2', target_bir_lowering=False, debug=False)
    x_d = nc.dram_tensor('x', [CIN, HW], bf16, kind='ExternalInput')
    w1_d = nc.dram_tensor('w1p', [128, 256], bf16, kind='ExternalInput')
    b1_d = nc.dram_tensor('bias1', [128, 1], f32, kind='ExternalInput')
    w2_d = nc.dram_tensor('w2p', [128, 9 * 128], bf16, kind='ExternalInput')
    b2_d = nc.dram_tensor('bias2', [128, 1], f32, kind='ExternalInput')
    w3_d = nc.dram_tensor('w3p', [128, 9 * 128], bf16, kind='ExternalInput')
    b3_d = nc.dram_tensor('bias3', [128, 1], f32, kind='ExternalInput')
    id_d = nc.dram_tensor('ident', [128, 128], bf16, kind='ExternalInput')
    ir_d = nc.dram_tensor('idxrel', [128, A * NBLK], bf16,
                          kind='ExternalInput')
    out_d = nc.dram_tensor('out', [128, HW], f32, kind='ExternalOutput')

    with tile.TileContext(nc) as tc, ExitStack() as st0:
        consts = st0.enter_context(tc.tile_pool(name='consts', bufs=1))
        h1t_pool = st0.enter_context(tc.tile_pool(name='h1t', bufs=1))
        pad_pool = st0.enter_context(tc.tile_pool(name='pads', bufs=1))
        outb_pool = st0.enter_context(tc.tile_pool(name='outb', bufs=3))

        # phase-1 critical DMAs first: conv1 weights/bias, the j-iota table
        # and the first idxrel slice (band-0/1 one-hot builds run on DVE
        # during the DMA-paced conv1); everything else queues behind the
        # first x slices or is deferred into the band phase
        w1_t = consts.tile([128, 2 * 128], bf16, tag='w1')
        nc.sync.dma_start(out=w1_t[:], in_=w1_d.ap())
        w2_t = consts.tile([128, 9 * 128], bf16, tag='w2')
        w3_t = consts.tile([128, 9 * 128], bf16, tag='w3')
        b1_t = consts.tile([128, 1], f32, tag='b1')
        b2_t = consts.tile([128, 1], f32, tag='b2')
        b3_t = consts.tile([128, 1], f32, tag='b3')
        nc.scalar.dma_start(out=b1_t[:], in_=b1_d.ap())
        id_t = consts.tile([128, 128], bf16, tag='ident')
        ir_t = consts.tile([128, A * NBLK], bf16, tag='idxrel')
        # j-iota table J2[p, j*NBLK+k] = j, generated on-chip as WMAX
        # constant memsets on DVE (exact in bf16).  Not on Pool: Pool's
        # in-order queue must start with the x-stream DMA issues.
        j2_t = consts.tile([128, WMAX * NBLK], bf16, tag='j2')
        for j in range(WMAX):
            nc.vector.memset(j2_t[:, j * NBLK:(j + 1) * NBLK], float(j))
        zero_t = consts.tile([128, 512], bf16, tag='zeros')
        nc.vector.memset(zero_t[:], 0.0)

        h1T = h1t_pool.tile([128, NBLK * 128], bf16, tag='h1T')

        dht_pad = pad_pool.tile([128, PADW * PADW], bf16, tag='dht_pad')
        h2_pad = pad_pool.tile([128, PADW * PADW], bf16, tag='h2_pad')
        def pad_border_memsets():
            # zero only the borders; the interior is fully overwritten.
            # Emitted after the conv1 loop so Pool's in-order queue serves
            # the x-stream DMA issues first (borders are needed only by
            # conv2/conv3, ~25us in).
            for pad_t in (dht_pad, h2_pad):
                pv = pad_t[:].rearrange('c (a r) -> c a r', a=PADW)
                nc.gpsimd.memset(pv[:, 0:1, :], 0.0)
                nc.gpsimd.memset(pv[:, PADW - 1:PADW, :], 0.0)
                nc.gpsimd.memset(pv[:, :, 0:1], 0.0)
                nc.gpsimd.memset(pv[:, :, PADW - 1:PADW], 0.0)

        with ExitStack() as stT:
            oh_pool = stT.enter_context(tc.tile_pool(name='oh', bufs=16))
            h1_pool = stT.enter_context(tc.tile_pool(name='h1', bufs=1))
            h1 = h1_pool.tile([128, NBLK * BPFULL], bf16, tag='h1')

            # ----------------------------------- conv1 + blockwise transposes
            # h1 [c, y, x] -> h1T: chunk b=(by,bx) holds pixels (dy,dx) on
            # partitions p=dy*10+dx, channels on columns.  Transposes of
            # block-row r-1 are emitted between conv1 slices so PE never
            # waits on the (ACT/DVE-paced) h1 epilogues; the epilogues and
            # pt drains alternate between ACT and DVE to halve either's
            # serial load during the DMA-paced conv1 phase.
            def build_oh(a):
                """DVE one-hot for angle a, (j,k) layout: [128, WMAX*NBLK]."""
                wa = int(WIN_A[a])
                oh = oh_pool.tile([128, WMAX * NBLK], bf16, tag='oh')
                ov = oh[:, :wa * NBLK].rearrange('p (j k) -> p j k', k=NBLK)
                jv = j2_t[:, :wa * NBLK].rearrange('p (j k) -> p j k', k=NBLK)
                irv = ir_t[:, a * NBLK:(a + 1) * NBLK].unsqueeze(
                    1).to_broadcast([128, wa, NBLK])
                nc.vector.tensor_tensor(out=ov, in0=jv, in1=irv,
                                        op=mybir.AluOpType.is_equal)
                return oh

            oh_ring = {}

            # h1 is stored pixel-BLOCKED with a uniform 1200-col stride per
            # block row (the 4-tall tail row only uses its first 400 cols):
            # col = (gy*10+gx)*120 + dy*10+dx.  Each transpose input is one
            # contiguous <=120-pixel chunk (walrus requires a single free
            # dim on the transpose src).
            hb = h1[:].rearrange('c (gy gx dy dx) -> c gy gx dy dx',
                                 gy=YBLK, gx=XBLK, dy=BH)
            with ExitStack() as st1:
                xf_pool = st1.enter_context(tc.tile_pool(name='xf', bufs=8))
                ps1 = st1.enter_context(
                    tc.tile_pool(name='ps1', bufs=2, space='PSUM'))
                pst = st1.enter_context(
                    tc.tile_pool(name='pst', bufs=3, space='PSUM'))
                CS = SROWS * W

                tp_cursor = [0]
                tp_flip = [0]

                def transpose_chunks(upto):
                    # transpose chunks in groups of <=8 (one 2KB psum bank
                    # as bf16), never mixing full and tail-row chunks;
                    # drains alternate ACT/DVE
                    while tp_cursor[0] + 8 <= upto or (upto == NBLK and
                                                       tp_cursor[0] < NBLK):
                        k0 = tp_cursor[0]
                        cut = ((YBLK - 1) * XBLK
                               if (BPTAIL != BPFULL
                                   and k0 < (YBLK - 1) * XBLK) else NBLK)
                        kc = min(8, cut - k0)
                        tp_cursor[0] = k0 + kc
                        bp = _chunk_bp(k0)
                        pt = pst.tile([128, 8 * 128], bf16, tag='pt',
                                      space='PSUM')
                        for kk in range(kc):
                            b = k0 + kk
                            nc.tensor.transpose(
                                out=pt[:bp, kk * 128:(kk + 1) * 128],
                                in_=h1[:, b * BPFULL:b * BPFULL + bp],
                                identity=id_t[:])
                        dst = h1T[:bp, k0 * 128:(k0 + kc) * 128]
                        tp_flip[0] += 1
                        if tp_flip[0] in (2, 6, 12):
                            nc.scalar.copy(out=dst, in_=pt[:bp, :kc * 128])
                        else:
                            nc.vector.tensor_scalar_add(
                                out=dst, in0=pt[:bp, :kc * 128],
                                scalar1=0.0)

                # one group = one block row; one DMA per ci-half
                ep = [0]
                for g in range(YBLK):
                    rows = BH if g < YBLK - 1 else _tail_rows()
                    gsl = slice(g * BH * W, g * BH * W + rows * W)
                    xfs = [xf_pool.tile([128, BH * W], bf16, tag=f'xf{hh}',
                                        name=f'xf{hh}_{g}')
                           for hh in range(2)]
                    if g < 2:
                        # split the first groups into per-slice pieces,
                        # slice-major, so the PE pipeline fills as soon as
                        # the first slice's two halves land
                        for s in range(rows // SROWS):
                            for hh in range(2):
                                (nc.sync, nc.gpsimd)[hh].dma_start(
                                    out=xfs[hh][:, s * CS:(s + 1) * CS],
                                    in_=x_d.ap()[hh * 128:(hh + 1) * 128,
                                                 g * BH * W + s * CS:
                                                 g * BH * W + (s + 1) * CS])
                    else:
                        for hh in range(2):
                            (nc.sync, nc.gpsimd)[hh].dma_start(
                                out=xfs[hh][:, :rows * W],
                                in_=x_d.ap()[hh * 128:(hh + 1) * 128, gsl])
                    if g == 0:
                        # small consts via the idle ACT queue (keeps the SP
                        # sequencer free for the x stream)
                        nc.scalar.dma_start(out=id_t[:], in_=id_d.ap())
                    if g == 1:
                        nc.scalar.dma_start(
                            out=ir_t[:, 0:BAND * NBLK],
                            in_=ir_d.ap()[:, 0:BAND * NBLK])
                        for a in range(BAND):
                            oh_ring[a] = build_oh(a)
                    if g == 5:
                        nc.sync.dma_start(
                            out=ir_t[:, BAND * NBLK:2 * BAND * NBLK],
                            in_=ir_d.ap()[:, BAND * NBLK:2 * BAND * NBLK])
                    if g == 6:
                        for a in range(BAND, 2 * BAND):
                            oh_ring[a] = build_oh(a)
                    for s in range(rows // SROWS):
                        ps = ps1.tile([128, CS], f32, tag='ps1')
                        for hh in range(2):
                            nc.tensor.matmul(
                                out=ps[:],
                                lhsT=w1_t[:, hh * 128:(hh + 1) * 128],
                                rhs=xfs[hh][:, s * CS:(s + 1) * CS],
                                start=(hh == 0), stop=(hh == 1))
                        # epilogue scatters row-major psum into blocked h1:
                        # psum col dy*100+gx*10+dx ->
                        #   h1 col gy*1200+gx*120+dy*10+dx
                        ob = hb[:, g:g + 1, :, s * SROWS:(s + 1) * SROWS, :]
                        ib = ps[:].rearrange('p (dy bx dx) -> p bx dy dx',
                                             dy=SROWS, bx=10).unsqueeze(1)
                        ep[0] += 1
                        if ep[0] not in (7, 14):
                            nc.scalar.activation(out=ob, in_=ib,
                                                 func=RELU, bias=b1_t[:, :1],
                                                 scale=1.0)
                        else:
                            # relu(x + b) on DVE: (in + bias) max 0
                            nc.vector.tensor_scalar(
                                out=ob, in0=ib, scalar1=b1_t[:, :1],
                                scalar2=0.0, op0=mybir.AluOpType.add,
                                op1=mybir.AluOpType.max)
                    if g >= 2:
                        # block rows <= g-1 are complete
                        transpose_chunks((g - 1) * XBLK)
                transpose_chunks(NBLK)

            pad_border_memsets()
            # stream the rest of the first idxrel quarter and the conv
            # weights once the x DMAs are queued
            nc.gpsimd.dma_start(
                out=ir_t[:, 2 * BAND * NBLK:25 * NBLK],
                in_=ir_d.ap()[:, 2 * BAND * NBLK:25 * NBLK])
            nc.sync.dma_start(out=w2_t[:], in_=w2_d.ap())
            nc.sync.dma_start(out=w3_t[:], in_=w3_d.ap())
            nc.scalar.dma_start(out=b2_t[:], in_=b2_d.ap())
            nc.scalar.dma_start(out=b3_t[:], in_=b3_d.ap())
            for a in range(2 * BAND, 3 * BAND):
                oh_ring[a] = build_oh(a)

            psd = stT.enter_context(
                tc.tile_pool(name='psd', bufs=5, space='PSUM'))
            psc = stT.enter_context(
                tc.tile_pool(name='psc', bufs=2, space='PSUM'))

            # -------------------------------------------------- DHT + convs
            def zero_bank(b, bank):
                # initialize the accumulator: all DHT matmuls use start=False
                # and accumulate onto zeroed PSUM.  The first banks are
                # zeroed by K=1 PE matmuls (DVE's in-order queue is clogged
                # with conv1 epilogues and one-hot builds at that point);
                # steady-state banks by DVE memset, off the critical PE path.
                if b < 1:
                    nc.tensor.matmul(out=bank[:, :BAND * R],
                                     lhsT=zero_t[:1, :128],
                                     rhs=zero_t[:1, :BAND * R], start=True,
                                     stop=False, skip_group_check=True)
                elif b == 1:
                    nc.scalar.activation(out=bank[:, :BAND * R],
                                         in_=zero_t[:, :BAND * R],
                                         func=COPY)
                else:
                    nc.vector.memset(bank[:, :BAND * R], 0.0)

            def dht_band(b, bank, ohs):
                for s in range(BAND):
                    a = b * BAND + s
                    ohvF = ohs[s][:BPFULL, :].rearrange(
                        'p (j k) -> p j k', k=NBLK)
                    ohvT = ohs[s][:BPTAIL, :].rearrange(
                        'p (j k) -> p j k', k=NBLK)
                    lo_a = LO[a]
                    win_a_ = WIN[a]
                    for k in range(NBLK):
                        lo = int(lo_a[k])
                        wk = int(win_a_[k])
                        bp = _chunk_bp(k)
                        ohv = ohvF if bp == BPFULL else ohvT
                        nc.tensor.matmul(
                            out=bank[:, s * R + lo:s * R + lo + wk],
                            lhsT=h1T[:bp, k * 128:(k + 1) * 128],
                            rhs=ohv[:, 0:wk, k:k + 1],
                            start=False, stop=False, skip_group_check=True)

            def drain_band(b, bank):
                a0 = b * BAND
                pv = bank[:, :BAND * R].rearrange('p (a r) -> p a r', a=BAND)
                dv = dht_pad[:].rearrange('c (a r) -> c a r', a=PADW)
                nc.scalar.activation(out=dv[:, a0 + 1:a0 + 1 + BAND, 1:1 + R],
                                     in_=pv[:], func=COPY)

            def conv2_band(c):
                a0 = c * BAND
                sv = dht_pad[:].rearrange('c (a r) -> c a r', a=PADW)
                ps = psc.tile([128, 512], f32, tag='conv')
                for t9 in range(9):
                    dy, dx = divmod(t9, 3)
                    nc.tensor.matmul(
                        out=ps[:, :BAND * R],
                        lhsT=w2_t[:, t9 * 128:(t9 + 1) * 128],
                        rhs=sv[:, a0 + dy:a0 + dy + BAND, dx:dx + R],
                        start=(t9 == 0), stop=(t9 == 8))
                pv = ps[:, :BAND * R].rearrange('p (a r) -> p a r', a=BAND)
                hv2 = h2_pad[:].rearrange('c (a r) -> c a r', a=PADW)
                nc.scalar.activation(out=hv2[:, a0 + 1:a0 + 1 + BAND, 1:1 + R],
                                     in_=pv[:], func=RELU, bias=b2_t[:, :1],
                                     scale=1.0)

            def conv3_band(c, ar=BAND, s0=0, dve_epi=False):
                a0 = c * BAND + s0
                sv = h2_pad[:].rearrange('c (a r) -> c a r', a=PADW)
                ps = psc.tile([128, 512], f32, tag='conv')
                for t9 in range(9):
                    dy, dx = divmod(t9, 3)
                    nc.tensor.matmul(
                        out=ps[:, :ar * R],
                        lhsT=w3_t[:, t9 * 128:(t9 + 1) * 128],
                        rhs=sv[:, a0 + dy:a0 + dy + ar, dx:dx + R],
                        start=(t9 == 0), stop=(t9 == 8))
                pv = ps[:, :ar * R].rearrange('p (a r) -> p a r', a=ar)
                ob = outb_pool.tile([128, ar * R], f32, tag=f'outb{ar}')
                ov = ob[:].rearrange('p (a r) -> p a r', a=ar)
                if dve_epi:
                    nc.vector.tensor_scalar(
                        out=ov[:], in0=pv[:], scalar1=b3_t[:, :1],
                        scalar2=0.0, op0=mybir.AluOpType.add,
                        op1=mybir.AluOpType.max)
                else:
                    nc.scalar.activation(out=ov[:], in_=pv[:], func=RELU,
                                         bias=b3_t[:, :1], scale=1.0)
                nc.sync.dma_start(out=out_d.ap()[:, a0 * R:(a0 + ar) * R],
                                  in_=ob[:])

            banks = {0: psd.tile([128, 512], f32, tag='band',
                                 name='bank_0')}
            zero_bank(0, banks[0])
            for b in range(NBAND):
                # stream the remaining idxrel quarters behind the builds
                if b in (2, 8, 14):
                    q = b // 6 + 1
                    nc.gpsimd.dma_start(
                        out=ir_t[:, q * 25 * NBLK:(q + 1) * 25 * NBLK],
                        in_=ir_d.ap()[:, q * 25 * NBLK:(q + 1) * 25 * NBLK])
                # build supply for band b+3
                if b + 3 < NBAND:
                    for s in range(BAND):
                        a = (b + 3) * BAND + s
                        oh_ring[a] = build_oh(a)
                if b + 1 < NBAND:
                    banks[b + 1] = psd.tile([128, 512], f32, tag='band',
                                            name=f'bank_{b + 1}')
                    zero_bank(b + 1, banks[b + 1])
                dht_band(b, banks[b],
                         [oh_ring[b * BAND + s] for s in range(BAND)])
                drain_band(b, banks[b])
                del banks[b]
                for s in range(BAND):
                    del oh_ring[b * BAND + s]
                if b >= 2:
                    conv2_band(b - 2)
                if b >= 4:
                    conv3_band(b - 4)
            for c in (NBAND - 2, NBAND - 1):
                conv2_band(c)
            for c in range(NBAND - 4, NBAND - 1):
                conv3_band(c)
            # split the last band so its epilogue/DMA pipeline with the
            # later pieces' matmuls instead of trailing the whole kernel
            conv3_band(NBAND - 1, ar=2, s0=0)
            conv3_band(NBAND - 1, ar=2, s0=2, dve_epi=True)

    if split_waits:
        _split_multi_waits(nc, mybir)
    _PROGRAM_CACHE[key] = nc
    return nc


# ----------------------------------------------------------------------------
# entry point
# ----------------------------------------------------------------------------

def make_in_maps(inputs):
    T = _dht_tables()
    x = np.asarray(inputs['x'], np.float32)
    w1p, bias1, w2p, bias2, w3p, bias3, ident = _prep_weights(
        *[np.asarray(inputs[k], np.float32) for k in
          ('w1', 'b1', 'g1', 'be1', 'm1', 'v1',
           'w2', 'b2', 'g2', 'be2', 'm2', 'v2',
           'w3', 'b3', 'g3', 'be3', 'm3', 'v3')])
    common = dict(w1p=w1p, bias1=bias1, w2p=w2p, bias2=bias2, w3p=w3p,
                  bias3=bias3, ident=ident, idxrel=T['idxrel'])
    return [
        {'x': np.ascontiguousarray(x[n]).reshape(CIN, HW).astype(BF16),
         **common}
        for n in range(N)
    ]


def run(inputs, trace=False):
    from concourse.bass_utils import run_bass_kernel_spmd

    nc = _build_program()
    in_maps = make_in_maps(inputs)
    res = run_bass_kernel_spmd(nc, in_maps, core_ids=list(range(N)),
                               trace=trace)
    out = np.stack([res.results[n]['out'].reshape(CMID, H, W)
                    for n in range(N)], axis=0)
    return out.astype(np.float32), res


def kernel(**inputs):
    out, _ = run(inputs, trace=False)
    return out



# revision 2
# speedup vs baseline: 1.2097x; 1.2097x over previous
"""Trainium2 Bass kernel for nn_DHT_Layer (conv1x1+BN+ReLU -> Deep Hough
Transform -> two 3x3 conv+BN+ReLU layers).

Sharding: data-parallel over batch. 8 images / 8 cores -> one image per core,
no collectives; full inputs in, full output out. Inside each core:
  conv1   : 1x1 conv in fp8e4 DoubleRow mode (K=256 per pass), weights split
            hi+lo e4m3 for accuracy -> 2 passes at 0.5 cyc/col (10k cyc
            total vs 20k bf16).  x is quantized e4m3 on host; the resulting
            per-element noise averages out in the DHT's ~100-pixel
            positive-sum bins (~0.4% end-to-end).  BN+ReLU epilogues write
            bf16 h1 in a pixel-blocked layout; PE transposes (bf16) scatter
            it to h1T with the PSUM->SBUF drains converting to fp8e4.
  DHT     : out[c,a,r] = sum_p h[c,p] * (idx[a,p]==r) as fp8 one-hot
            matmuls.  Pixels are chunked 10x10 (100 chunks of 100 pixels);
            per angle, chunks are PAIRED along the direction that minimizes
            the rho-window union (x/y/diag/antidiag) and each pair runs as
            one DoubleRow matmul (two K=100 slices, 0.5 cyc/col over the
            union window); leftovers run as plain fp8 matmuls.  Total
            ~28.8k cyc-equivalents vs 90.7k bf16 baseline.  The fp8
            one-hots are precomputed on host (geometry-only) and streamed
            per 4-angle band via DMA (~14 MB, ~31 us, fully overlapped).
  conv2/3 : 3x3 convs as 9 shifted bf16 matmuls over a zero-padded
            [c, 102*102] layout, BN+ReLU folded into the epilogue;
            interleaved into the DHT band loop (conv2 lags 2 bands, conv3
            lags 4) so PE never waits on an ACT drain it just requested.

The local walrus build only supports ONE sync-wait per instruction, so a
post-pass splits multi-wait instructions into single-wait NoOp carriers.
"""

import functools
import math

import ml_dtypes
import numpy as np

N = 8          # batch / cores
CIN = 256
CMID = 128
H = W = 100
HW = H * W
A = 100        # angles
R = 100        # rho bins
BH, BW = 10, 10
YBLK = 10
XBLK = 10
NBLK = YBLK * XBLK             # 100 chunks
BP = BH * BW                   # 100 pixels per chunk (contraction dim)
SROWS = 5      # conv1 slice height (rows per psum chunk)
BAND = 4       # angles per PSUM bank / conv2 row band
NBAND = A // BAND
PADW = W + 2   # 102 padded spatial for 3x3 convs
BN_EPS = 1e-5
BF16 = ml_dtypes.bfloat16
FP8 = ml_dtypes.float8_e4m3


# ----------------------------------------------------------------------------
# host-side precomputation (shapes are fixed -> cache)
# ----------------------------------------------------------------------------

def _hough_idx():
    irho = int(math.sqrt(H * H + W * W) + 1) / float(R)
    theta = np.arange(A) * (math.pi / A)
    tab_cos = np.cos(theta) / irho
    tab_sin = np.sin(theta) / irho
    yy, xx = np.meshgrid(np.arange(H) - H // 2, np.arange(W) - W // 2,
                         indexing='ij')
    r = np.round(xx[None, :, :] * tab_cos[:, None, None]
                 + yy[None, :, :] * tab_sin[:, None, None])
    return np.clip(r + R // 2, 0, R - 1).astype(np.int32)  # [A, H, W]


@functools.lru_cache(maxsize=1)
def _dht_plan():
    """Per-angle DoubleRow pairing plan + host-built fp8 one-hot table.

    Returns dict with:
      entries[a]: list of ('p', k1, k2, lo_u, win_u, coloff) and
                  ('s', k, lo, win, coloff); coloff is absolute into ohtab.
      band_off[b]: first ohtab column of band b (b in 0..NBAND, sentinel).
      ohtab: [BP, TOT] fp8 one-hot table.
      ohmax: max columns of any band.
    """
    idx = _hough_idx()
    lo = np.zeros((A, NBLK), np.int64)
    hi = np.zeros((A, NBLK), np.int64)
    # pix[k, p] = (y, x) of partition p in chunk k
    sub_idx = np.zeros((A, NBLK, BP), np.int64)
    for gy in range(YBLK):
        for gx in range(XBLK):
            k = gy * XBLK + gx
            sub = idx[:, gy * BH:(gy + 1) * BH,
                      gx * BW:(gx + 1) * BW].reshape(A, BP)
            sub_idx[:, k] = sub
            lo[:, k] = sub.min(axis=1)
            hi[:, k] = sub.max(axis=1)
    win = hi - lo + 1

    def pairing(a, d):
        dy, dx = d
        used = np.zeros((YBLK, XBLK), bool)
        pairs, singles = [], []
        for gy in range(YBLK):
            for gx in range(XBLK):
                if used[gy, gx]:
                    continue
                gy2, gx2 = gy + dy, gx + dx
                if (0 <= gy2 < YBLK and 0 <= gx2 < XBLK
                        and not used[gy2, gx2]):
                    used[gy, gx] = used[gy2, gx2] = True
                    pairs.append((gy * XBLK + gx, gy2 * XBLK + gx2))
                else:
                    used[gy, gx] = True
                    singles.append(gy * XBLK + gx)
        cost = sum(0.5 * (max(hi[a, k1], hi[a, k2])
                          - min(lo[a, k1], lo[a, k2]) + 1)
                   for k1, k2 in pairs)
        cost += sum(float(win[a, k]) for k in singles)
        return cost, pairs, singles

    entries = []
    cols = []          # per-column data appended as (a-local structure)
    band_off = [0]
    ohcols = []        # list of np arrays [BP] per column
    for b in range(NBAND):
        for s in range(BAND):
            a = b * BAND + s
            best = None
            for d in ((0, 1), (1, 0), (1, 1), (1, -1)):
                c, p, sg = pairing(a, d)
                if best is None or c < best[0]:
                    best = (c, p, sg)
            _, pairs, singles = best
            ents = []
            for k1, k2 in pairs:
                lo_u = int(min(lo[a, k1], lo[a, k2]))
                win_u = int(max(hi[a, k1], hi[a, k2])) - lo_u + 1
                coloff = len(ohcols)
                for k in (k1, k2):
                    rel = sub_idx[a, k] - lo_u          # [BP]
                    oh = np.zeros((win_u, BP), np.float32)
                    oh[rel, np.arange(BP)] = 1.0
                    for j in range(win_u):
                        ohcols.append(oh[j])
                ents.append(('p', k1, k2, lo_u, win_u, coloff))
            for k in singles:
                lo_s = int(lo[a, k]); win_s = int(win[a, k])
                coloff = len(ohcols)
                rel = sub_idx[a, k] - lo_s
                oh = np.zeros((win_s, BP), np.float32)
                oh[rel, np.arange(BP)] = 1.0
                for j in range(win_s):
                    ohcols.append(oh[j])
                ents.append(('s', k, lo_s, win_s, coloff))
            entries.append(ents)
        band_off.append(len(ohcols))
    ohtab = np.ascontiguousarray(
        np.stack(ohcols, axis=1).astype(FP8))       # [BP, TOT]
    ohmax = max(band_off[i + 1] - band_off[i] for i in range(NBAND))
    return dict(entries=entries, band_off=band_off, ohtab=ohtab,
                ohmax=ohmax)


def _q8(x):
    return x.astype(FP8).astype(np.float32)


def _prep_weights(w1, b1, g1, be1, m1, v1, w2, b2, g2, be2, m2, v2,
                  w3, b3, g3, be3, m3, v3):
    s1 = g1 / np.sqrt(v1 + BN_EPS)
    s2 = g2 / np.sqrt(v2 + BN_EPS)
    s3 = g3 / np.sqrt(v3 + BN_EPS)
    # conv1: y[co] = sum_ci w1[co,ci]*x[ci]; fold BN scale into co rows.
    # fp8 hi/lo split; layout [ci%128, grp*256 + half*128 + co].
    w1f = (w1[:, :, 0, 0] * s1[:, None]).T            # [ci=256, co=128]
    w1h = w1f.reshape(2, 128, 128).transpose(1, 0, 2)  # [ci128, half, co]
    w1_hi = _q8(w1h)
    w1_lo = (w1h - w1_hi)
    w1p8 = np.ascontiguousarray(
        np.stack([w1_hi, w1_lo], axis=1)               # [ci128, grp, half, co]
        .reshape(128, 512).astype(FP8))
    bias1 = ((b1 - m1) * s1 + be1).astype(np.float32).reshape(128, 1)
    # conv2/3: DRAM layout [ci, tap*128+co] -> one contiguous-row DMA.
    w2f = (w2 * s2[:, None, None, None]).transpose(2, 3, 1, 0)  # [ky,kx,ci,co]
    w2p = np.ascontiguousarray(
        w2f.reshape(9, 128, 128).transpose(1, 0, 2).reshape(128, 9 * 128)
        .astype(BF16))
    bias2 = ((b2 - m2) * s2 + be2).astype(np.float32).reshape(128, 1)
    w3f = (w3 * s3[:, None, None, None]).transpose(2, 3, 1, 0)
    w3p = np.ascontiguousarray(
        w3f.reshape(9, 128, 128).transpose(1, 0, 2).reshape(128, 9 * 128)
        .astype(BF16))
    bias3 = ((b3 - m3) * s3 + be3).astype(np.float32).reshape(128, 1)
    ident = np.eye(128, dtype=BF16)
    return w1p8, bias1, w2p, bias2, w3p, bias3, ident


# ----------------------------------------------------------------------------
# walrus workaround: split multi-wait instructions (this build supports only
# one sync-wait per instruction)
# ----------------------------------------------------------------------------

def _split_multi_waits(nc, mybir, max_waits=1):
    cnt = 0
    for f in nc.m.functions:
        for bb in f.blocks:
            insts = list(bb.instructions)
            new = []
            changed = False
            for inst in insts:
                si = inst.sync_info
                if si is not None:
                    ow = list(si.on_wait)
                    if len(ow) > max_waits:
                        changed = True
                        head = ow[:-max_waits]
                        for i in range(0, len(head), max_waits):
                            nop = mybir.InstNoOp(name=f'waitsplit_{cnt}',
                                                 ins=[], outs=[])
                            cnt += 1
                            nop.engine = inst.engine
                            nop.sync_info = mybir.SyncInfo(
                                on_wait=head[i:i + max_waits], on_update=[])
                            new.append(nop)
                        si.on_wait = ow[-max_waits:]
                new.append(inst)
            if changed:
                bb.instructions = new
    return cnt


# ----------------------------------------------------------------------------
# bass program
# ----------------------------------------------------------------------------

_PROGRAM_CACHE = {}


def _build_program(split_waits=True):
    key = ('nc', split_waits)
    if key in _PROGRAM_CACHE:
        return _PROGRAM_CACHE[key]
    import concourse.bass as bass
    import concourse.mybir as mybir
    import concourse.tile as tile
    from concourse.ap import AP
    from contextlib import ExitStack

    plan = _dht_plan()
    ENTRIES = plan['entries']
    BAND_OFF = plan['band_off']
    OHMAX = plan['ohmax']
    OHTOT = BAND_OFF[-1]

    f32 = mybir.dt.float32
    bf16 = mybir.dt.bfloat16
    fp8 = mybir.dt.float8e4
    RELU = mybir.ActivationFunctionType.Relu
    COPY = mybir.ActivationFunctionType.Copy
    DR = mybir.MatmulPerfMode.DoubleRow

    nc = bass.Bass('TRN2', target_bir_lowering=False, debug=False)
    x_d = nc.dram_tensor('x', [CIN, HW], fp8, kind='ExternalInput')
    w1_d = nc.dram_tensor('w1p8', [128, 512], fp8, kind='ExternalInput')
    b1_d = nc.dram_tensor('bias1', [128, 1], f32, kind='ExternalInput')
    w2_d = nc.dram_tensor('w2p', [128, 9 * 128], bf16, kind='ExternalInput')
    b2_d = nc.dram_tensor('bias2', [128, 1], f32, kind='ExternalInput')
    w3_d = nc.dram_tensor('w3p', [128, 9 * 128], bf16, kind='ExternalInput')
    b3_d = nc.dram_tensor('bias3', [128, 1], f32, kind='ExternalInput')
    id_d = nc.dram_tensor('ident', [128, 128], bf16, kind='ExternalInput')
    oh_d = nc.dram_tensor('ohtab', [BP, OHTOT], fp8, kind='ExternalInput')
    out_d = nc.dram_tensor('out', [128, HW], f32, kind='ExternalOutput')

    def dr_ap(base_ap, offset, istride, icount, nstride, ncount, parts):
        """3-dim AP [parts, icount, ncount] for DoubleRow operands."""
        return AP(base_ap.tensor, base_ap.offset + offset,
                  [[base_ap.ap[0][0], parts],
                   [istride, icount], [nstride, ncount]])

    with tile.TileContext(nc) as tc, ExitStack() as st0:
        consts = st0.enter_context(tc.tile_pool(name='consts', bufs=1))
        h1t_pool = st0.enter_context(tc.tile_pool(name='h1t', bufs=1))
        pad_pool = st0.enter_context(tc.tile_pool(name='pads', bufs=1))
        outb_pool = st0.enter_context(tc.tile_pool(name='outb', bufs=3))
        oh_pool = st0.enter_context(tc.tile_pool(name='oh', bufs=3))

        w1_t = consts.tile([128, 512], fp8, tag='w1')
        nc.sync.dma_start(out=w1_t[:], in_=w1_d.ap())
        w2_t = consts.tile([128, 9 * 128], bf16, tag='w2')
        w3_t = consts.tile([128, 9 * 128], bf16, tag='w3')
        b1_t = consts.tile([128, 1], f32, tag='b1')
        b2_t = consts.tile([128, 1], f32, tag='b2')
        b3_t = consts.tile([128, 1], f32, tag='b3')
        nc.scalar.dma_start(out=b1_t[:], in_=b1_d.ap())
        id_t = consts.tile([128, 128], bf16, tag='ident')
        zero_t = consts.tile([128, 512], bf16, tag='zeros')
        nc.vector.memset(zero_t[:], 0.0)

        h1T = h1t_pool.tile([128, NBLK * 128], fp8, tag='h1T')

        oh_tiles = {}

        def issue_oh(b, eng):
            t = oh_pool.tile([128, OHMAX], fp8, tag='oh', name=f'oh_{b}')
            cols = BAND_OFF[b + 1] - BAND_OFF[b]
            eng.dma_start(out=t[:BP, :cols],
                          in_=oh_d.ap()[0:BP, BAND_OFF[b]:BAND_OFF[b + 1]])
            oh_tiles[b] = t

        dht_pad = pad_pool.tile([128, PADW * PADW], bf16, tag='dht_pad')
        h2_pad = pad_pool.tile([128, PADW * PADW], bf16, tag='h2_pad')

        def pad_border_memsets():
            # zero only the borders; the interior is fully overwritten.
            for pad_t in (dht_pad, h2_pad):
                pv = pad_t[:].rearrange('c (a r) -> c a r', a=PADW)
                nc.gpsimd.memset(pv[:, 0:1, :], 0.0)
                nc.gpsimd.memset(pv[:, PADW - 1:PADW, :], 0.0)
                nc.gpsimd.memset(pv[:, :, 0:1], 0.0)
                nc.gpsimd.memset(pv[:, :, PADW - 1:PADW], 0.0)

        with ExitStack() as stT:
            h1_pool = stT.enter_context(tc.tile_pool(name='h1', bufs=1))
            h1 = h1_pool.tile([128, NBLK * BP], bf16, tag='h1')

            # ----------------------------------- conv1 + blockwise transposes
            # h1 [c, y, x] -> h1T: chunk b=(by,bx) holds pixels (dy,dx) on
            # partitions p=dy*10+dx, channels on columns (fp8 after drain).
            hb = h1[:].rearrange('c (gy gx dy dx) -> c gy gx dy dx',
                                 gy=YBLK, gx=XBLK, dy=BH)
            with ExitStack() as st1:
                xf_pool = st1.enter_context(tc.tile_pool(name='xf', bufs=4))
                ps1 = st1.enter_context(
                    tc.tile_pool(name='ps1', bufs=2, space='PSUM'))
                pst = st1.enter_context(
                    tc.tile_pool(name='pst', bufs=3, space='PSUM'))
                CS = SROWS * W

                tp_cursor = [0]
                tp_flip = [0]

                def transpose_chunks(upto):
                    # transpose chunks in groups of 8 (one 2KB psum bank as
                    # bf16); drains alternate ACT/DVE and convert to fp8
                    while tp_cursor[0] + 8 <= upto or (upto == NBLK and
                                                       tp_cursor[0] < NBLK):
                        k0 = tp_cursor[0]
                        kc = min(8, NBLK - k0)
                        tp_cursor[0] = k0 + kc
                        pt = pst.tile([128, 8 * 128], bf16, tag='pt',
                                      space='PSUM')
                        for kk in range(kc):
                            b = k0 + kk
                            nc.tensor.transpose(
                                out=pt[:BP, kk * 128:(kk + 1) * 128],
                                in_=h1[:, b * BP:(b + 1) * BP],
                                identity=id_t[:])
                        dst = h1T[:BP, k0 * 128:(k0 + kc) * 128]
                        tp_flip[0] += 1
                        if tp_flip[0] in (2, 6, 12):
                            nc.scalar.copy(out=dst, in_=pt[:BP, :kc * 128])
                        else:
                            nc.vector.tensor_scalar_add(
                                out=dst, in0=pt[:BP, :kc * 128],
                                scalar1=0.0)

                # one group = one block row; one DMA per ci-half into a
                # single tile (halves side by side for DoubleRow rhs)
                ep = [0]
                for g in range(YBLK):
                    gsl = slice(g * BH * W, (g + 1) * BH * W)
                    xf = xf_pool.tile([128, 2 * BH * W], fp8, tag='xf',
                                      name=f'xf_{g}')
                    if g < 2:
                        # split the first groups into per-slice pieces,
                        # slice-major, so the PE pipeline fills as soon as
                        # the first slice's two halves land
                        for s in range(BH // SROWS):
                            for hh in range(2):
                                (nc.sync, nc.gpsimd)[hh].dma_start(
                                    out=xf[:, hh * BH * W + s * CS:
                                           hh * BH * W + (s + 1) * CS],
                                    in_=x_d.ap()[hh * 128:(hh + 1) * 128,
                                                 g * BH * W + s * CS:
                                                 g * BH * W + (s + 1) * CS])
                    else:
                        for hh in range(2):
                            (nc.sync, nc.gpsimd)[hh].dma_start(
                                out=xf[:, hh * BH * W:(hh + 1) * BH * W],
                                in_=x_d.ap()[hh * 128:(hh + 1) * 128, gsl])
                    if g == 0:
                        # small consts via the idle ACT queue (keeps the SP
                        # sequencer free for the x stream)
                        nc.scalar.dma_start(out=id_t[:], in_=id_d.ap())
                    if g == 2:
                        issue_oh(0, nc.scalar)
                    if g == 5:
                        issue_oh(1, nc.scalar)
                    if g == 8:
                        issue_oh(2, nc.scalar)
                    for s in range(BH // SROWS):
                        ps = ps1.tile([128, CS], f32, tag='ps1')
                        for grp in range(2):
                            lhsT = dr_ap(w1_t[:], grp * 256, 128, 2, 1, 128,
                                         128)
                            rhs = dr_ap(xf[:], s * CS, BH * W, 2, 1, CS, 128)
                            nc.tensor.matmul(
                                out=ps[:], lhsT=lhsT, rhs=rhs,
                                start=(grp == 0), stop=(grp == 1),
                                perf_mode=DR)
                        # epilogue scatters row-major psum into blocked h1:
                        # psum col dy*100+gx*10+dx ->
                        #   h1 col gy*1000+gx*100+dy*10+dx
                        ob = hb[:, g:g + 1, :, s * SROWS:(s + 1) * SROWS, :]
                        ib = ps[:].rearrange('p (dy bx dx) -> p bx dy dx',
                                             dy=SROWS, bx=10).unsqueeze(1)
                        ep[0] += 1
                        if ep[0] not in (7, 14):
                            nc.scalar.activation(out=ob, in_=ib,
                                                 func=RELU, bias=b1_t[:, :1],
                                                 scale=1.0)
                        else:
                            # relu(x + b) on DVE: (in + bias) max 0
                            nc.vector.tensor_scalar(
                                out=ob, in0=ib, scalar1=b1_t[:, :1],
                                scalar2=0.0, op0=mybir.AluOpType.add,
                                op1=mybir.AluOpType.max)
                    if g >= 2:
                        # block rows <= g-1 are complete
                        transpose_chunks((g - 1) * XBLK)
                transpose_chunks(NBLK)

            pad_border_memsets()
            # stream the conv weights once the x DMAs are queued
            nc.sync.dma_start(out=w2_t[:], in_=w2_d.ap())
            nc.sync.dma_start(out=w3_t[:], in_=w3_d.ap())
            nc.scalar.dma_start(out=b2_t[:], in_=b2_d.ap())
            nc.scalar.dma_start(out=b3_t[:], in_=b3_d.ap())

            psd = stT.enter_context(
                tc.tile_pool(name='psd', bufs=5, space='PSUM'))
            psc = stT.enter_context(
                tc.tile_pool(name='psc', bufs=2, space='PSUM'))

            # -------------------------------------------------- DHT + convs
            def zero_bank(b, bank):
                # initialize the accumulator: all DHT matmuls use start=False
                # and accumulate onto zeroed PSUM.
                if b < 1:
                    nc.tensor.matmul(out=bank[:, :BAND * R],
                                     lhsT=zero_t[:1, :128],
                                     rhs=zero_t[:1, :BAND * R], start=True,
                                     stop=False, skip_group_check=True)
                elif b == 1:
                    nc.scalar.activation(out=bank[:, :BAND * R],
                                         in_=zero_t[:, :BAND * R],
                                         func=COPY)
                else:
                    nc.vector.memset(bank[:, :BAND * R], 0.0)

            def dht_band(b, bank):
                oh = oh_tiles[b]
                off0 = BAND_OFF[b]
                for s in range(BAND):
                    a = b * BAND + s
                    for ent in ENTRIES[a]:
                        if ent[0] == 'p':
                            _, k1, k2, lo_u, win_u, coloff = ent
                            lhsT = dr_ap(h1T[:], k1 * 128,
                                         (k2 - k1) * 128, 2, 1, 128, BP)
                            rhs = dr_ap(oh[:], coloff - off0,
                                        win_u, 2, 1, win_u, BP)
                            nc.tensor.matmul(
                                out=bank[:, s * R + lo_u:
                                         s * R + lo_u + win_u],
                                lhsT=lhsT, rhs=rhs, start=False, stop=False,
                                skip_group_check=True, perf_mode=DR)
                        else:
                            _, k, lo_s, win_s, coloff = ent
                            c0 = coloff - off0
                            nc.tensor.matmul(
                                out=bank[:, s * R + lo_s:
                                         s * R + lo_s + win_s],
                                lhsT=h1T[:BP, k * 128:(k + 1) * 128],
                                rhs=oh[:BP, c0:c0 + win_s],
                                start=False, stop=False,
                                skip_group_check=True)

            def drain_band(b, bank):
                a0 = b * BAND
                pv = bank[:, :BAND * R].rearrange('p (a r) -> p a r', a=BAND)
                dv = dht_pad[:].rearrange('c (a r) -> c a r', a=PADW)
                nc.scalar.activation(out=dv[:, a0 + 1:a0 + 1 + BAND, 1:1 + R],
                                     in_=pv[:], func=COPY)

            def conv2_band(c):
                a0 = c * BAND
                sv = dht_pad[:].rearrange('c (a r) -> c a r', a=PADW)
                ps = psc.tile([128, 512], f32, tag='conv')
                for t9 in range(9):
                    dy, dx = divmod(t9, 3)
                    nc.tensor.matmul(
                        out=ps[:, :BAND * R],
                        lhsT=w2_t[:, t9 * 128:(t9 + 1) * 128],
                        rhs=sv[:, a0 + dy:a0 + dy + BAND, dx:dx + R],
                        start=(t9 == 0), stop=(t9 == 8))
                pv = ps[:, :BAND * R].rearrange('p (a r) -> p a r', a=BAND)
                hv2 = h2_pad[:].rearrange('c (a r) -> c a r', a=PADW)
                nc.scalar.activation(out=hv2[:, a0 + 1:a0 + 1 + BAND, 1:1 + R],
                                     in_=pv[:], func=RELU, bias=b2_t[:, :1],
                                     scale=1.0)

            def conv3_band(c, ar=BAND, s0=0, dve_epi=False):
                a0 = c * BAND + s0
                sv = h2_pad[:].rearrange('c (a r) -> c a r', a=PADW)
                ps = psc.tile([128, 512], f32, tag='conv')
                for t9 in range(9):
                    dy, dx = divmod(t9, 3)
                    nc.tensor.matmul(
                        out=ps[:, :ar * R],
                        lhsT=w3_t[:, t9 * 128:(t9 + 1) * 128],
                        rhs=sv[:, a0 + dy:a0 + dy + ar, dx:dx + R],
                        start=(t9 == 0), stop=(t9 == 8))
                pv = ps[:, :ar * R].rearrange('p (a r) -> p a r', a=ar)
                ob = outb_pool.tile([128, ar * R], f32, tag=f'outb{ar}')
                ov = ob[:].rearrange('p (a r) -> p a r', a=ar)
                if dve_epi:
                    nc.vector.tensor_scalar(
                        out=ov[:], in0=pv[:], scalar1=b3_t[:, :1],
                        scalar2=0.0, op0=mybir.AluOpType.add,
                        op1=mybir.AluOpType.max)
                else:
                    nc.scalar.activation(out=ov[:], in_=pv[:], func=RELU,
                                         bias=b3_t[:, :1], scale=1.0)
                nc.sync.dma_start(out=out_d.ap()[:, a0 * R:(a0 + ar) * R],
                                  in_=ob[:])

            banks = {0: psd.tile([128, 512], f32, tag='band',
                                 name='bank_0')}
            zero_bank(0, banks[0])
            for b in range(NBAND):
                # stream the upcoming one-hot bands behind the PE
                if b + 3 < NBAND:
                    issue_oh(b + 3, (nc.sync, nc.gpsimd, nc.scalar)[b % 3])
                if b + 1 < NBAND:
                    banks[b + 1] = psd.tile([128, 512], f32, tag='band',
                                            name=f'bank_{b + 1}')
                    zero_bank(b + 1, banks[b + 1])
                dht_band(b, banks[b])
                drain_band(b, banks[b])
                del banks[b]
                del oh_tiles[b]
                if b >= 2:
                    conv2_band(b - 2)
                if b >= 4:
                    conv3_band(b - 4)
            for c in (NBAND - 2, NBAND - 1):
                conv2_band(c)
            for c in range(NBAND - 4, NBAND - 1):
                conv3_band(c)
            # split the last band so its epilogue/DMA pipeline with the
            # later pieces' matmuls instead of trailing the whole kernel
            conv3_band(NBAND - 1, ar=2, s0=0)
            conv3_band(NBAND - 1, ar=2, s0=2, dve_epi=True)

    if split_waits:
        _split_multi_waits(nc, mybir)
    _PROGRAM_CACHE[key] = nc
    return nc


# ----------------------------------------------------------------------------
# entry point
# ----------------------------------------------------------------------------

def make_in_maps(inputs):
    plan = _dht_plan()
    x = np.asarray(inputs['x'], np.float32)
    w1p8, bias1, w2p, bias2, w3p, bias3, ident = _prep_weights(
        *[np.asarray(inputs[k], np.float32) for k in
          ('w1', 'b1', 'g1', 'be1', 'm1', 'v1',
           'w2', 'b2', 'g2', 'be2', 'm2', 'v2',
           'w3', 'b3', 'g3', 'be3', 'm3', 'v3')])
    common = dict(w1p8=w1p8, bias1=bias1, w2p=w2p, bias2=bias2, w3p=w3p,
                  bias3=bias3, ident=ident, ohtab=plan['ohtab'])
    return [
        {'x': np.ascontiguousarray(x[n]).reshape(CIN, HW).astype(FP8),
         **common}
        for n in range(N)
    ]


def run(inputs, trace=False):
    from concourse.bass_utils import run_bass_kernel_spmd

    nc = _build_program()
    in_maps = make_in_maps(inputs)
    res = run_bass_kernel_spmd(nc, in_maps, core_ids=list(range(N)),
                               trace=trace)
    out = np.stack([res.results[n]['out'].reshape(CMID, H, W)
                    for n in range(N)], axis=0)
    return out.astype(np.float32), res


def kernel(**inputs):
    out, _ = run(inputs, trace=False)
    return out


# revision 19
# speedup vs baseline: 1.4033x; 1.1600x over previous
"""Trainium2 Bass kernel for nn_DHT_Layer (conv1x1+BN+ReLU -> Deep Hough
Transform -> two 3x3 conv+BN+ReLU layers).

Sharding: data-parallel over batch. 8 images / 8 cores -> one image per core,
no collectives; full inputs in, full output out. Inside each core:
  conv1   : 1x1 conv in fp8e4 DoubleRow mode (K=256 per pass), weights split
            hi+lo e4m3 for accuracy -> 2 passes at 0.5 cyc/col (10k cyc
            total vs 20k bf16).  x is quantized e4m3 on host; the resulting
            per-element noise averages out in the DHT's ~100-pixel
            positive-sum bins (~0.4% end-to-end).  BN+ReLU epilogues write
            bf16 h1 in a pixel-blocked layout; PE transposes (bf16) scatter
            it to h1T with the PSUM->SBUF drains converting to fp8e4.
  DHT     : out[c,a,r] = sum_p h[c,p] * (idx[a,p]==r) as fp8 one-hot
            matmuls.  Pixels are chunked 10x10 (100 chunks of 100 pixels);
            per angle, chunks are PAIRED along the direction that minimizes
            the rho-window union (x/y/diag/antidiag) and each pair runs as
            one DoubleRow matmul (two K=100 slices, 0.5 cyc/col over the
            union window); leftovers run as plain fp8 matmuls.  Total
            ~28.8k cyc-equivalents vs 90.7k bf16 baseline.  The fp8
            one-hots are precomputed on host (geometry-only) and streamed
            per 4-angle band via DMA (~14 MB, ~31 us, fully overlapped).
  conv2/3 : 3x3 convs as 9 shifted bf16 matmuls over a zero-padded
            [c, 102*102] layout, BN+ReLU folded into the epilogue;
            interleaved into the DHT band loop (conv2 lags 2 bands, conv3
            lags 4) so PE never waits on an ACT drain it just requested.

The local walrus build only supports ONE sync-wait per instruction, so a
post-pass splits multi-wait instructions into single-wait NoOp carriers.
"""

import functools
import math

import ml_dtypes
import numpy as np

N = 8          # batch / cores
CIN = 256
CMID = 128
H = W = 100
HW = H * W
A = 100        # angles
R = 100        # rho bins
BH, BW = 10, 10
YBLK = 10
XBLK = 10
NBLK = YBLK * XBLK             # 100 chunks
BP = BH * BW                   # 100 pixels per chunk (contraction dim)
SROWS = 5      # conv1 slice height (rows per psum chunk)
BAND = 4       # angles per PSUM bank / conv2 row band
NBAND = A // BAND
PADW = W + 2   # 102 padded spatial for 3x3 convs
BN_EPS = 1e-5
BF16 = ml_dtypes.bfloat16
FP8 = ml_dtypes.float8_e4m3
# power-of-two activation scales keep fp8 in range (DHT sums reach ~920,
# conv2+BN outputs ~1700); exact in fp8, inverses folded into w2/w3
SC1 = 2.0 ** -4     # h1T / DHT domain
SC2 = 2.0 ** -5     # h2 / conv3-input domain


# ----------------------------------------------------------------------------
# host-side precomputation (shapes are fixed -> cache)
# ----------------------------------------------------------------------------

def _hough_idx():
    irho = int(math.sqrt(H * H + W * W) + 1) / float(R)
    theta = np.arange(A) * (math.pi / A)
    tab_cos = np.cos(theta) / irho
    tab_sin = np.sin(theta) / irho
    yy, xx = np.meshgrid(np.arange(H) - H // 2, np.arange(W) - W // 2,
                         indexing='ij')
    r = np.round(xx[None, :, :] * tab_cos[:, None, None]
                 + yy[None, :, :] * tab_sin[:, None, None])
    return np.clip(r + R // 2, 0, R - 1).astype(np.int32)  # [A, H, W]


@functools.lru_cache(maxsize=1)
def _dht_plan():
    """Per-angle DoubleRow pairing plan + host-built fp8 one-hot table.

    Returns dict with:
      entries[a]: list of ('p', k1, k2, lo_u, win_u, coloff) and
                  ('s', k, lo, win, coloff); coloff is absolute into ohtab.
      band_off[b]: first ohtab column of band b (b in 0..NBAND, sentinel).
      ohtab: [BP, TOT] fp8 one-hot table.
      ohmax: max columns of any band.
    """
    idx = _hough_idx()
    lo = np.zeros((A, NBLK), np.int64)
    hi = np.zeros((A, NBLK), np.int64)
    # pix[k, p] = (y, x) of partition p in chunk k
    sub_idx = np.zeros((A, NBLK, BP), np.int64)
    for gy in range(YBLK):
        for gx in range(XBLK):
            k = gy * XBLK + gx
            sub = idx[:, gy * BH:(gy + 1) * BH,
                      gx * BW:(gx + 1) * BW].reshape(A, BP)
            sub_idx[:, k] = sub
            lo[:, k] = sub.min(axis=1)
            hi[:, k] = sub.max(axis=1)
    win = hi - lo + 1

    def pairing(a, d):
        dy, dx = d
        used = np.zeros((YBLK, XBLK), bool)
        pairs, singles = [], []
        for gy in range(YBLK):
            for gx in range(XBLK):
                if used[gy, gx]:
                    continue
                gy2, gx2 = gy + dy, gx + dx
                if (0 <= gy2 < YBLK and 0 <= gx2 < XBLK
                        and not used[gy2, gx2]):
                    used[gy, gx] = used[gy2, gx2] = True
                    pairs.append((gy * XBLK + gx, gy2 * XBLK + gx2))
                else:
                    used[gy, gx] = True
                    singles.append(gy * XBLK + gx)
        cost = sum(0.5 * (max(hi[a, k1], hi[a, k2])
                          - min(lo[a, k1], lo[a, k2]) + 1)
                   for k1, k2 in pairs)
        cost += sum(float(win[a, k]) for k in singles)
        return cost, pairs, singles

    entries = []
    cols = []          # per-column data appended as (a-local structure)
    band_off = [0]
    ohcols = []        # list of np arrays [BP] per column
    for b in range(NBAND):
        for s in range(BAND):
            a = b * BAND + s
            best = None
            for d in ((0, 1), (1, 0), (1, 1), (1, -1)):
                c, p, sg = pairing(a, d)
                if best is None or c < best[0]:
                    best = (c, p, sg)
            _, pairs, singles = best
            ents = []
            for k1, k2 in pairs:
                lo_u = int(min(lo[a, k1], lo[a, k2]))
                win_u = int(max(hi[a, k1], hi[a, k2])) - lo_u + 1
                coloff = len(ohcols)
                for k in (k1, k2):
                    rel = sub_idx[a, k] - lo_u          # [BP]
                    oh = np.zeros((win_u, BP), np.float32)
                    oh[rel, np.arange(BP)] = 1.0
                    for j in range(win_u):
                        ohcols.append(oh[j])
                ents.append(('p', k1, k2, lo_u, win_u, coloff))
            for k in singles:
                lo_s = int(lo[a, k]); win_s = int(win[a, k])
                coloff = len(ohcols)
                rel = sub_idx[a, k] - lo_s
                oh = np.zeros((win_s, BP), np.float32)
                oh[rel, np.arange(BP)] = 1.0
                for j in range(win_s):
                    ohcols.append(oh[j])
                ents.append(('s', k, lo_s, win_s, coloff))
            entries.append(ents)
        band_off.append(len(ohcols))
    ohtab = np.ascontiguousarray(
        np.stack(ohcols, axis=1).astype(FP8))       # [BP, TOT]
    ohmax = max(band_off[i + 1] - band_off[i] for i in range(NBAND))
    return dict(entries=entries, band_off=band_off, ohtab=ohtab,
                ohmax=ohmax)


def _q8(x):
    return x.astype(FP8).astype(np.float32)


def _prep_weights(w1, b1, g1, be1, m1, v1, w2, b2, g2, be2, m2, v2,
                  w3, b3, g3, be3, m3, v3):
    s1 = g1 / np.sqrt(v1 + BN_EPS)
    s2 = g2 / np.sqrt(v2 + BN_EPS)
    s3 = g3 / np.sqrt(v3 + BN_EPS)
    # conv1: y[co] = sum_ci w1[co,ci]*x[ci]; fold BN scale into co rows.
    # fp8 hi/lo split; layout [ci%128, grp*256 + half*128 + co].
    w1f = (w1[:, :, 0, 0] * s1[:, None]).T            # [ci=256, co=128]
    w1h = w1f.reshape(2, 128, 128).transpose(1, 0, 2)  # [ci128, half, co]
    w1_hi = _q8(w1h)
    w1_lo = (w1h - w1_hi)
    w1p8 = np.ascontiguousarray(
        np.stack([w1_hi, w1_lo], axis=1)               # [ci128, grp, half, co]
        .reshape(128, 512).astype(FP8))
    bias1 = ((b1 - m1) * s1 + be1).astype(np.float32).reshape(128, 1)

    # conv2/3: fp8 hi/lo split, layout [ci, hi(9*128) | lo(9*128) | zero(128)]
    def conv_w8(w, s, inv_in_scale):
        wf = (w * s[:, None, None, None]).transpose(2, 3, 1, 0)  # [ky,kx,ci,co]
        wf = wf.reshape(9, 128, 128).transpose(1, 0, 2)          # [ci,tap,co]
        wf = wf * inv_in_scale
        w_hi = _q8(wf)
        w_lo = wf - w_hi
        # [zero(128) | hi(1152) | lo(1152)]
        arr = np.zeros((128, 2432), np.float32)
        arr[:, 128:1280] = w_hi.reshape(128, 1152)
        arr[:, 1280:2432] = w_lo.reshape(128, 1152)
        return np.ascontiguousarray(arr.astype(FP8))

    w2p8 = conv_w8(w2, s2, 1.0 / SC1)
    bias2 = (((b2 - m2) * s2 + be2) * SC2).astype(np.float32).reshape(128, 1)
    w3p8 = conv_w8(w3, s3, 1.0 / SC2)
    bias3 = ((b3 - m3) * s3 + be3).astype(np.float32).reshape(128, 1)
    ident = np.eye(128, dtype=BF16)
    return w1p8, bias1, w2p8, bias2, w3p8, bias3, ident


# ----------------------------------------------------------------------------
# walrus workaround: split multi-wait instructions (this build supports only
# one sync-wait per instruction)
# ----------------------------------------------------------------------------

def _split_multi_waits(nc, mybir, max_waits=1):
    cnt = 0
    for f in nc.m.functions:
        for bb in f.blocks:
            insts = list(bb.instructions)
            new = []
            changed = False
            for inst in insts:
                si = inst.sync_info
                if si is not None:
                    ow = list(si.on_wait)
                    if len(ow) > max_waits:
                        changed = True
                        head = ow[:-max_waits]
                        for i in range(0, len(head), max_waits):
                            nop = mybir.InstNoOp(name=f'waitsplit_{cnt}',
                                                 ins=[], outs=[])
                            cnt += 1
                            nop.engine = inst.engine
                            nop.sync_info = mybir.SyncInfo(
                                on_wait=head[i:i + max_waits], on_update=[])
                            new.append(nop)
                        si.on_wait = ow[-max_waits:]
                new.append(inst)
            if changed:
                bb.instructions = new
    return cnt


# ----------------------------------------------------------------------------
# bass program
# ----------------------------------------------------------------------------

_PROGRAM_CACHE = {}


def _build_program(split_waits=True):
    key = ('nc', split_waits)
    if key in _PROGRAM_CACHE:
        return _PROGRAM_CACHE[key]
    import concourse.bass as bass
    import concourse.mybir as mybir
    import concourse.tile as tile
    from concourse.ap import AP
    from contextlib import ExitStack

    plan = _dht_plan()
    ENTRIES = plan['entries']
    BAND_OFF = plan['band_off']
    OHMAX = plan['ohmax']
    OHTOT = BAND_OFF[-1]

    f32 = mybir.dt.float32
    bf16 = mybir.dt.bfloat16
    fp8 = mybir.dt.float8e4
    RELU = mybir.ActivationFunctionType.Relu
    COPY = mybir.ActivationFunctionType.Copy
    DR = mybir.MatmulPerfMode.DoubleRow

    nc = bass.Bass('TRN2', target_bir_lowering=False, debug=False)
    x_d = nc.dram_tensor('x', [CIN, HW], fp8, kind='ExternalInput')
    w1_d = nc.dram_tensor('w1p8', [128, 512], fp8, kind='ExternalInput')
    b1_d = nc.dram_tensor('bias1', [128, 1], f32, kind='ExternalInput')
    w2_d = nc.dram_tensor('w2p8', [128, 2432], fp8, kind='ExternalInput')
    b2_d = nc.dram_tensor('bias2', [128, 1], f32, kind='ExternalInput')
    w3_d = nc.dram_tensor('w3p8', [128, 2432], fp8, kind='ExternalInput')
    b3_d = nc.dram_tensor('bias3', [128, 1], f32, kind='ExternalInput')
    id_d = nc.dram_tensor('ident', [128, 128], bf16, kind='ExternalInput')
    oh_d = nc.dram_tensor('ohtab', [BP, OHTOT], fp8, kind='ExternalInput')
    out_d = nc.dram_tensor('out', [128, HW], f32, kind='ExternalOutput')

    def dr_ap(base_ap, offset, istride, icount, nstride, ncount, parts):
        """3-dim AP [parts, icount, ncount] for DoubleRow operands."""
        return AP(base_ap.tensor, base_ap.offset + offset,
                  [[base_ap.ap[0][0], parts],
                   [istride, icount], [nstride, ncount]])

    with tile.TileContext(nc) as tc, ExitStack() as st0:
        consts = st0.enter_context(tc.tile_pool(name='consts', bufs=1))
        h1t_pool = st0.enter_context(tc.tile_pool(name='h1t', bufs=1))
        pad_pool = st0.enter_context(tc.tile_pool(name='pads', bufs=1))
        outb_pool = st0.enter_context(tc.tile_pool(name='outb', bufs=3))
        oh_pool = st0.enter_context(tc.tile_pool(name='oh', bufs=3))

        w1_t = consts.tile([128, 512], fp8, tag='w1')
        nc.sync.dma_start(out=w1_t[:], in_=w1_d.ap())
        w2_t = consts.tile([128, 2432], fp8, tag='w2')
        w3_t = consts.tile([128, 2432], fp8, tag='w3')
        b1_t = consts.tile([128, 1], f32, tag='b1')
        b2_t = consts.tile([128, 1], f32, tag='b2')
        b3_t = consts.tile([128, 1], f32, tag='b3')
        nc.scalar.dma_start(out=b1_t[:], in_=b1_d.ap())
        id_t = consts.tile([128, 128], bf16, tag='ident')
        zero_t = consts.tile([128, 512], bf16, tag='zeros')
        nc.vector.memset(zero_t[:], 0.0)

        h1T = h1t_pool.tile([128, NBLK * 128], fp8, tag='h1T')

        oh_tiles = {}

        def issue_oh(b, eng):
            t = oh_pool.tile([128, OHMAX], fp8, tag='oh', name=f'oh_{b}')
            cols = BAND_OFF[b + 1] - BAND_OFF[b]
            eng.dma_start(out=t[:BP, :cols],
                          in_=oh_d.ap()[0:BP, BAND_OFF[b]:BAND_OFF[b + 1]])
            oh_tiles[b] = t

        # conv inputs as fp8 hi/lo pairs in ONE tile each ([lo | hi] halves
        # so all DoubleRow pair strides stay positive); h2_pad keeps the
        # bf16 conv2 output so lo = relu(psum+b) - hi is computable.
        HOFF = PADW * PADW
        dpad8 = pad_pool.tile([128, 2 * HOFF], fp8, tag='dpad8')
        hpad8 = pad_pool.tile([128, 2 * HOFF], fp8, tag='hpad8')
        h2_pad = pad_pool.tile([128, PADW * PADW], bf16, tag='h2_pad')

        def pad_border_memsets():
            # zero only the borders; the interior is fully overwritten.
            for pad_t in (dpad8, hpad8):
                pv = pad_t[:].rearrange('c (two a r) -> c two a r', two=2,
                                        a=PADW)
                nc.gpsimd.memset(pv[:, :, 0:1, :], 0.0)
                nc.gpsimd.memset(pv[:, :, PADW - 1:PADW, :], 0.0)
                nc.gpsimd.memset(pv[:, :, :, 0:1], 0.0)
                nc.gpsimd.memset(pv[:, :, :, PADW - 1:PADW], 0.0)

        with ExitStack() as stT:
            h1_pool = stT.enter_context(tc.tile_pool(name='h1', bufs=1))
            h1 = h1_pool.tile([128, NBLK * BP], bf16, tag='h1')

            # ----------------------------------- conv1 + blockwise transposes
            # h1 [c, y, x] -> h1T: chunk b=(by,bx) holds pixels (dy,dx) on
            # partitions p=dy*10+dx, channels on columns (fp8 after drain).
            hb = h1[:].rearrange('c (gy gx dy dx) -> c gy gx dy dx',
                                 gy=YBLK, gx=XBLK, dy=BH)
            with ExitStack() as st1:
                xf_pool = st1.enter_context(tc.tile_pool(name='xf', bufs=4))
                ps1 = st1.enter_context(
                    tc.tile_pool(name='ps1', bufs=2, space='PSUM'))
                pst = st1.enter_context(
                    tc.tile_pool(name='pst', bufs=3, space='PSUM'))
                CS = SROWS * W

                tp_cursor = [0]
                tp_flip = [0]

                def transpose_chunks(upto):
                    # transpose chunks in groups of 8 (one 2KB psum bank as
                    # bf16); drains alternate ACT/DVE and convert to fp8
                    while tp_cursor[0] + 8 <= upto or (upto == NBLK and
                                                       tp_cursor[0] < NBLK):
                        k0 = tp_cursor[0]
                        kc = min(8, NBLK - k0)
                        tp_cursor[0] = k0 + kc
                        pt = pst.tile([128, 8 * 128], bf16, tag='pt',
                                      space='PSUM')
                        for kk in range(kc):
                            b = k0 + kk
                            nc.tensor.transpose(
                                out=pt[:BP, kk * 128:(kk + 1) * 128],
                                in_=h1[:, b * BP:(b + 1) * BP],
                                identity=id_t[:])
                        dst = h1T[:BP, k0 * 128:(k0 + kc) * 128]
                        tp_flip[0] += 1
                        if tp_flip[0] in (2, 6, 12):
                            nc.scalar.activation(
                                out=dst, in_=pt[:BP, :kc * 128],
                                func=COPY, scale=SC1)
                        else:
                            nc.vector.tensor_scalar(
                                out=dst, in0=pt[:BP, :kc * 128],
                                scalar1=SC1, scalar2=None,
                                op0=mybir.AluOpType.mult)

                # one group = one block row; one DMA per ci-half into a
                # single tile (halves side by side for DoubleRow rhs)
                ep = [0]
                for g in range(YBLK):
                    gsl = slice(g * BH * W, (g + 1) * BH * W)
                    xf = xf_pool.tile([128, 2 * BH * W], fp8, tag='xf',
                                      name=f'xf_{g}')
                    if g < 2:
                        # split the first groups into per-slice pieces,
                        # slice-major, so the PE pipeline fills as soon as
                        # the first slice's two halves land
                        for s in range(BH // SROWS):
                            for hh in range(2):
                                (nc.sync, nc.gpsimd)[hh].dma_start(
                                    out=xf[:, hh * BH * W + s * CS:
                                           hh * BH * W + (s + 1) * CS],
                                    in_=x_d.ap()[hh * 128:(hh + 1) * 128,
                                                 g * BH * W + s * CS:
                                                 g * BH * W + (s + 1) * CS])
                    else:
                        for hh in range(2):
                            (nc.sync, nc.gpsimd)[hh].dma_start(
                                out=xf[:, hh * BH * W:(hh + 1) * BH * W],
                                in_=x_d.ap()[hh * 128:(hh + 1) * 128, gsl])
                    if g == 0:
                        # small consts via the idle ACT queue (keeps the SP
                        # sequencer free for the x stream)
                        nc.scalar.dma_start(out=id_t[:], in_=id_d.ap())
                    if g == 2:
                        issue_oh(0, nc.scalar)
                    if g == 5:
                        issue_oh(1, nc.scalar)
                    if g == 8:
                        issue_oh(2, nc.scalar)
                    for s in range(BH // SROWS):
                        ps = ps1.tile([128, CS], f32, tag='ps1')
                        for grp in range(2):
                            lhsT = dr_ap(w1_t[:], grp * 256, 128, 2, 1, 128,
                                         128)
                            rhs = dr_ap(xf[:], s * CS, BH * W, 2, 1, CS, 128)
                            nc.tensor.matmul(
                                out=ps[:], lhsT=lhsT, rhs=rhs,
                                start=(grp == 0), stop=(grp == 1),
                                perf_mode=DR)
                        # epilogue scatters row-major psum into blocked h1:
                        # psum col dy*100+gx*10+dx ->
                        #   h1 col gy*1000+gx*100+dy*10+dx
                        ob = hb[:, g:g + 1, :, s * SROWS:(s + 1) * SROWS, :]
                        ib = ps[:].rearrange('p (dy bx dx) -> p bx dy dx',
                                             dy=SROWS, bx=10).unsqueeze(1)
                        ep[0] += 1
                        if ep[0] not in (7, 14):
                            nc.scalar.activation(out=ob, in_=ib,
                                                 func=RELU, bias=b1_t[:, :1],
                                                 scale=1.0)
                        else:
                            # relu(x + b) on DVE: (in + bias) max 0
                            nc.vector.tensor_scalar(
                                out=ob, in0=ib, scalar1=b1_t[:, :1],
                                scalar2=0.0, op0=mybir.AluOpType.add,
                                op1=mybir.AluOpType.max)
                    if g >= 2:
                        # block rows <= g-1 are complete
                        transpose_chunks((g - 1) * XBLK)
                transpose_chunks(NBLK)

            pad_border_memsets()
            # stream the conv weights once the x DMAs are queued
            nc.sync.dma_start(out=w2_t[:], in_=w2_d.ap())
            nc.sync.dma_start(out=w3_t[:], in_=w3_d.ap())
            nc.scalar.dma_start(out=b2_t[:], in_=b2_d.ap())
            nc.scalar.dma_start(out=b3_t[:], in_=b3_d.ap())

            psd = stT.enter_context(
                tc.tile_pool(name='psd', bufs=4, space='PSUM'))
            psc = stT.enter_context(
                tc.tile_pool(name='psc', bufs=2, space='PSUM'))

            # -------------------------------------------------- DHT + convs
            def zero_bank(b, bank):
                # initialize the accumulator: all DHT matmuls use start=False
                # and accumulate onto zeroed PSUM.
                if b < 1:
                    nc.tensor.matmul(out=bank[:, :BAND * R],
                                     lhsT=zero_t[:1, :128],
                                     rhs=zero_t[:1, :BAND * R], start=True,
                                     stop=False, skip_group_check=True)
                elif b == 1:
                    nc.scalar.activation(out=bank[:, :BAND * R],
                                         in_=zero_t[:, :BAND * R],
                                         func=COPY)
                else:
                    nc.vector.memset(bank[:, :BAND * R], 0.0)

            def dht_band(b, bank):
                oh = oh_tiles[b]
                off0 = BAND_OFF[b]
                for s in range(BAND):
                    a = b * BAND + s
                    for ent in ENTRIES[a]:
                        if ent[0] == 'p':
                            _, k1, k2, lo_u, win_u, coloff = ent
                            lhsT = dr_ap(h1T[:], k1 * 128,
                                         (k2 - k1) * 128, 2, 1, 128, BP)
                            rhs = dr_ap(oh[:], coloff - off0,
                                        win_u, 2, 1, win_u, BP)
                            nc.tensor.matmul(
                                out=bank[:, s * R + lo_u:
                                         s * R + lo_u + win_u],
                                lhsT=lhsT, rhs=rhs, start=False, stop=False,
                                skip_group_check=True, perf_mode=DR)
                        else:
                            _, k, lo_s, win_s, coloff = ent
                            c0 = coloff - off0
                            nc.tensor.matmul(
                                out=bank[:, s * R + lo_s:
                                         s * R + lo_s + win_s],
                                lhsT=h1T[:BP, k * 128:(k + 1) * 128],
                                rhs=oh[:BP, c0:c0 + win_s],
                                start=False, stop=False,
                                skip_group_check=True)

            def pad_views(pad_t, a0, na):
                """(lo, hi) interior views [c, na, R] of a hi/lo pad tile."""
                pv = pad_t[:].rearrange('c (two a r) -> c two a r', two=2,
                                        a=PADW)
                lo = pv[:, 0:1, a0 + 1:a0 + 1 + na, 1:1 + R].squeeze(1)
                hi = pv[:, 1:2, a0 + 1:a0 + 1 + na, 1:1 + R].squeeze(1)
                return lo, hi

            def drain_band(b, bank):
                # psum -> fp8 hi (ACT) + fp8 lo residual (DVE subtract)
                a0 = b * BAND
                pv = bank[:, :BAND * R].rearrange('p (a r) -> p a r', a=BAND)
                lo_v, hi_v = pad_views(dpad8, a0, BAND)
                nc.scalar.activation(out=hi_v, in_=pv[:], func=COPY)
                nc.vector.tensor_tensor(out=lo_v, in0=pv[:], in1=hi_v,
                                        op=mybir.AluOpType.subtract)

            # conv psum layout: per band one [128, 1024] tile = 2 zero
            # regions; sub-band s2 (2 angles) accumulates over the FLATTENED
            # padded rows at cols [s2*512, s2*512+202) (junk at row-boundary
            # cols, skipped by the epilogue's PADW-strided reads).
            LSUB = PADW + R    # 202

            def emit14(ps, s2, pad_t, w_t, a0):
                """3-product fp8 conv: 14 DoubleRow passes accumulating
                w_hi*x_hi + w_hi*x_lo + w_lo*x_hi over 9 taps."""
                def wc(g, t):
                    return 128 + g * 1152 + t * 128

                def xc(h, t):
                    return (h * HOFF + (t // 3) * PADW + (t % 3)
                            + a0 * PADW)

                pairs = []
                for t in (0, 2, 4, 6):
                    pairs.append(((wc(0, t), xc(1, t)),
                                  (wc(0, t + 1), xc(1, t + 1))))
                for t in (0, 2, 4, 6):
                    pairs.append(((wc(0, t), xc(0, t)),
                                  (wc(0, t + 1), xc(0, t + 1))))
                for t in (0, 2, 4, 6):
                    pairs.append(((wc(1, t), xc(1, t)),
                                  (wc(1, t + 1), xc(1, t + 1))))
                pairs.append(((wc(0, 8), xc(0, 8)), (wc(1, 8), xc(1, 8))))
                # odd 27th product pairs with the zero-weight block (w col
                # 0); the dummy rhs member re-reads tap-0's valid window
                pairs.append(((0, xc(1, 0)), (wc(0, 8), xc(1, 8))))
                wa = w_t[:]
                xa = pad_t[:]
                out_v = ps[:, s2 * 512:s2 * 512 + LSUB]
                for i, ((w1c, x1c), (w2c, x2c)) in enumerate(pairs):
                    lhsT = AP(wa.tensor, wa.offset + w1c,
                              [[wa.ap[0][0], 128], [w2c - w1c, 2], [1, 128]])
                    rhs = AP(xa.tensor, xa.offset + x1c,
                             [[xa.ap[0][0], 128], [x2c - x1c, 2], [1, LSUB]])
                    nc.tensor.matmul(out=out_v, lhsT=lhsT, rhs=rhs,
                                     start=(i == 0),
                                     stop=(i == len(pairs) - 1),
                                     perf_mode=DR)

            def conv_psum_view(ps, ar, s0=0):
                """[p, (sub, al), R] view of the padded conv psum."""
                pa = ps[:]
                if ar == BAND:
                    return AP(pa.tensor, pa.offset,
                              [[pa.ap[0][0], 128], [512, 2], [PADW, 2],
                               [1, R]])
                return AP(pa.tensor, pa.offset,
                          [[pa.ap[0][0], 128], [PADW, ar], [1, R]])

            def conv2_band(c):
                a0 = c * BAND
                ps = psc.tile([128, 1024], f32, tag='conv')
                for s2 in range(2):
                    emit14(ps, s2, dpad8, w2_t, a0 + s2 * 2)
                pv = conv_psum_view(ps, BAND)
                hv2 = h2_pad[:].rearrange('c (a r) -> c a r', a=PADW)
                h2v = hv2[:, a0 + 1:a0 + 1 + BAND, 1:1 + R]
                h2v4 = h2v.rearrange('c (x y) r -> c x y r', x=2)
                nc.scalar.activation(out=h2v4, in_=pv, func=RELU,
                                     bias=b2_t[:, :1], scale=SC2)
                lo_v, hi_v = pad_views(hpad8, a0, BAND)
                nc.scalar.activation(out=hi_v, in_=h2v, func=COPY)
                nc.vector.tensor_tensor(out=lo_v, in0=h2v, in1=hi_v,
                                        op=mybir.AluOpType.subtract)

            def conv3_band(c, ar=BAND, s0=0, dve_epi=False):
                a0 = c * BAND + s0
                ps = psc.tile([128, 1024], f32, tag='conv')
                for s2 in range(ar // 2):
                    emit14(ps, s2, hpad8, w3_t, a0 + s2 * 2)
                pv = conv_psum_view(ps, ar)
                ob = outb_pool.tile([128, ar * R], f32, tag=f'outb{ar}')
                if ar == BAND:
                    ov = ob[:].rearrange('p (x y r) -> p x y r', x=2, y=2)
                else:
                    ov = ob[:].rearrange('p (a r) -> p a r', a=ar)
                if dve_epi:
                    nc.vector.tensor_scalar(
                        out=ov, in0=pv, scalar1=b3_t[:, :1],
                        scalar2=0.0, op0=mybir.AluOpType.add,
                        op1=mybir.AluOpType.max)
                else:
                    nc.scalar.activation(out=ov, in_=pv, func=RELU,
                                         bias=b3_t[:, :1], scale=1.0)
                nc.sync.dma_start(out=out_d.ap()[:, a0 * R:(a0 + ar) * R],
                                  in_=ob[:])

            banks = {0: psd.tile([128, 512], f32, tag='band',
                                 name='bank_0')}
            zero_bank(0, banks[0])
            for b in range(NBAND):
                # stream the upcoming one-hot bands behind the PE
                if b + 3 < NBAND:
                    issue_oh(b + 3, (nc.sync, nc.gpsimd, nc.scalar)[b % 3])
                if b + 1 < NBAND:
                    banks[b + 1] = psd.tile([128, 512], f32, tag='band',
                                            name=f'bank_{b + 1}')
                    zero_bank(b + 1, banks[b + 1])
                dht_band(b, banks[b])
                drain_band(b, banks[b])
                del banks[b]
                del oh_tiles[b]
                if b >= 2:
                    conv2_band(b - 2)
                if b >= 4:
                    conv3_band(b - 4)
            for c in (NBAND - 2, NBAND - 1):
                conv2_band(c)
            for c in range(NBAND - 4, NBAND - 1):
                conv3_band(c)
            # split the last band so its epilogue/DMA pipeline with the
            # later pieces' matmuls instead of trailing the whole kernel
            conv3_band(NBAND - 1, ar=2, s0=0)
            conv3_band(NBAND - 1, ar=2, s0=2, dve_epi=True)

    if split_waits:
        _split_multi_waits(nc, mybir)
    _PROGRAM_CACHE[key] = nc
    return nc


# ----------------------------------------------------------------------------
# entry point
# ----------------------------------------------------------------------------

def make_in_maps(inputs):
    plan = _dht_plan()
    x = np.asarray(inputs['x'], np.float32)
    w1p8, bias1, w2p8, bias2, w3p8, bias3, ident = _prep_weights(
        *[np.asarray(inputs[k], np.float32) for k in
          ('w1', 'b1', 'g1', 'be1', 'm1', 'v1',
           'w2', 'b2', 'g2', 'be2', 'm2', 'v2',
           'w3', 'b3', 'g3', 'be3', 'm3', 'v3')])
    common = dict(w1p8=w1p8, bias1=bias1, w2p8=w2p8, bias2=bias2, w3p8=w3p8,
                  bias3=bias3, ident=ident, ohtab=plan['ohtab'])
    return [
        {'x': np.ascontiguousarray(x[n]).reshape(CIN, HW).astype(FP8),
         **common}
        for n in range(N)
    ]


def run(inputs, trace=False):
    from concourse.bass_utils import run_bass_kernel_spmd

    nc = _build_program()
    in_maps = make_in_maps(inputs)
    res = run_bass_kernel_spmd(nc, in_maps, core_ids=list(range(N)),
                               trace=trace)
    out = np.stack([res.results[n]['out'].reshape(CMID, H, W)
                    for n in range(N)], axis=0)
    return out.astype(np.float32), res


def kernel(**inputs):
    out, _ = run(inputs, trace=False)
    return out


# revision 57
# speedup vs baseline: 1.5243x; 1.0862x over previous
"""Trainium2 Bass kernel for nn_DHT_Layer (conv1x1+BN+ReLU -> Deep Hough
Transform -> two 3x3 conv+BN+ReLU layers).

Sharding: data-parallel over batch. 8 images / 8 cores -> one image per core,
no collectives; full inputs in, full output out. Inside each core:
  conv1   : 1x1 conv in fp8e4 DoubleRow mode (K=256 per pass), weights split
            hi+lo e4m3 for accuracy -> 2 passes at 0.5 cyc/col (10k cyc
            total vs 20k bf16).  x is quantized e4m3 on host; the resulting
            per-element noise averages out in the DHT's ~100-pixel
            positive-sum bins (~0.4% end-to-end).  BN+ReLU epilogues write
            bf16 h1 in a pixel-blocked layout; PE transposes (bf16) scatter
            it to h1T with the PSUM->SBUF drains converting to fp8e4.
  DHT     : out[c,a,r] = sum_p h[c,p] * (idx[a,p]==r) as fp8 one-hot
            matmuls.  Pixels are chunked 10x10 (100 chunks of 100 pixels);
            per angle, chunks are PAIRED along the direction that minimizes
            the rho-window union (x/y/diag/antidiag) and each pair runs as
            one DoubleRow matmul (two K=100 slices, 0.5 cyc/col over the
            union window); leftovers run as plain fp8 matmuls.  Total
            ~28.8k cyc-equivalents vs 90.7k bf16 baseline.  The fp8
            one-hots are precomputed on host (geometry-only) and streamed
            per 4-angle band via DMA (~14 MB, ~31 us, fully overlapped).
  conv2/3 : 3x3 convs as 9 shifted bf16 matmuls over a zero-padded
            [c, 102*102] layout, BN+ReLU folded into the epilogue;
            interleaved into the DHT band loop (conv2 lags 2 bands, conv3
            lags 4) so PE never waits on an ACT drain it just requested.

The local walrus build only supports ONE sync-wait per instruction, so a
post-pass splits multi-wait instructions into single-wait NoOp carriers.
"""

import functools
import math

import ml_dtypes
import numpy as np

N = 8          # batch / cores
CIN = 256
CMID = 128
H = W = 100
HW = H * W
A = 100        # angles
R = 100        # rho bins
BH, BW = 10, 10
YBLK = 10
XBLK = 10
NBLK = YBLK * XBLK             # 100 chunks
BP = BH * BW                   # 100 pixels per chunk (contraction dim)
SROWS = 5      # conv1 slice height (rows per psum chunk)
BAND = 4       # angles per PSUM bank / conv2 row band
NBAND = A // BAND
PADW = W + 2   # 102 padded spatial for 3x3 convs
BN_EPS = 1e-5
BF16 = ml_dtypes.bfloat16
FP8 = ml_dtypes.float8_e4m3
# power-of-two activation scales keep fp8 in range (DHT sums reach ~920,
# conv2+BN outputs ~1700); exact in fp8, inverses folded into w2/w3
SC1 = 2.0 ** -4     # h1T / DHT domain
SC2 = 2.0 ** -5     # h2 / conv3-input domain


# ----------------------------------------------------------------------------
# host-side precomputation (shapes are fixed -> cache)
# ----------------------------------------------------------------------------

def _hough_idx():
    irho = int(math.sqrt(H * H + W * W) + 1) / float(R)
    theta = np.arange(A) * (math.pi / A)
    tab_cos = np.cos(theta) / irho
    tab_sin = np.sin(theta) / irho
    yy, xx = np.meshgrid(np.arange(H) - H // 2, np.arange(W) - W // 2,
                         indexing='ij')
    r = np.round(xx[None, :, :] * tab_cos[:, None, None]
                 + yy[None, :, :] * tab_sin[:, None, None])
    return np.clip(r + R // 2, 0, R - 1).astype(np.int32)  # [A, H, W]


@functools.lru_cache(maxsize=1)
def _dht_plan():
    """Per-angle DoubleRow pairing plan + host-built fp8 one-hot table.

    Returns dict with:
      entries[a]: list of ('p', k1, k2, lo_u, win_u, coloff) and
                  ('s', k, lo, win, coloff); coloff is absolute into ohtab.
      band_off[b]: first ohtab column of band b (b in 0..NBAND, sentinel).
      ohtab: [BP, TOT] fp8 one-hot table.
      ohmax: max columns of any band.
    """
    idx = _hough_idx()
    lo = np.zeros((A, NBLK), np.int64)
    hi = np.zeros((A, NBLK), np.int64)
    # pix[k, p] = (y, x) of partition p in chunk k
    sub_idx = np.zeros((A, NBLK, BP), np.int64)
    for gy in range(YBLK):
        for gx in range(XBLK):
            k = gy * XBLK + gx
            sub = idx[:, gy * BH:(gy + 1) * BH,
                      gx * BW:(gx + 1) * BW].reshape(A, BP)
            sub_idx[:, k] = sub
            lo[:, k] = sub.min(axis=1)
            hi[:, k] = sub.max(axis=1)
    win = hi - lo + 1

    def pairing(a, d):
        dy, dx = d
        used = np.zeros((YBLK, XBLK), bool)
        pairs, singles = [], []
        for gy in range(YBLK):
            for gx in range(XBLK):
                if used[gy, gx]:
                    continue
                gy2, gx2 = gy + dy, gx + dx
                if (0 <= gy2 < YBLK and 0 <= gx2 < XBLK
                        and not used[gy2, gx2]):
                    used[gy, gx] = used[gy2, gx2] = True
                    pairs.append((gy * XBLK + gx, gy2 * XBLK + gx2))
                else:
                    used[gy, gx] = True
                    singles.append(gy * XBLK + gx)
        cost = sum(0.5 * (max(hi[a, k1], hi[a, k2])
                          - min(lo[a, k1], lo[a, k2]) + 1)
                   for k1, k2 in pairs)
        cost += sum(float(win[a, k]) for k in singles)
        return cost, pairs, singles

    entries = []
    cols = []          # per-column data appended as (a-local structure)
    band_off = [0]
    ohcols = []        # list of np arrays [BP] per column
    for b in range(NBAND):
        for s in range(BAND):
            a = b * BAND + s
            best = None
            for d in ((0, 1), (1, 0), (1, 1), (1, -1)):
                c, p, sg = pairing(a, d)
                if best is None or c < best[0]:
                    best = (c, p, sg)
            _, pairs, singles = best
            ents = []
            for k1, k2 in pairs:
                lo_u = int(min(lo[a, k1], lo[a, k2]))
                win_u = int(max(hi[a, k1], hi[a, k2])) - lo_u + 1
                coloff = len(ohcols)
                for k in (k1, k2):
                    rel = sub_idx[a, k] - lo_u          # [BP]
                    oh = np.zeros((win_u, BP), np.float32)
                    oh[rel, np.arange(BP)] = 1.0
                    for j in range(win_u):
                        ohcols.append(oh[j])
                ents.append(('p', k1, k2, lo_u, win_u, coloff))
            for k in singles:
                lo_s = int(lo[a, k]); win_s = int(win[a, k])
                coloff = len(ohcols)
                rel = sub_idx[a, k] - lo_s
                oh = np.zeros((win_s, BP), np.float32)
                oh[rel, np.arange(BP)] = 1.0
                for j in range(win_s):
                    ohcols.append(oh[j])
                ents.append(('s', k, lo_s, win_s, coloff))
            entries.append(ents)
        band_off.append(len(ohcols))
    ohtab = np.ascontiguousarray(
        np.stack(ohcols, axis=1).astype(FP8))       # [BP, TOT]
    ohmax = max(band_off[i + 1] - band_off[i] for i in range(NBAND))
    return dict(entries=entries, band_off=band_off, ohtab=ohtab,
                ohmax=ohmax)


def _q8(x):
    return x.astype(FP8).astype(np.float32)


def _prep_weights(w1, b1, g1, be1, m1, v1, w2, b2, g2, be2, m2, v2,
                  w3, b3, g3, be3, m3, v3):
    s1 = g1 / np.sqrt(v1 + BN_EPS)
    s2 = g2 / np.sqrt(v2 + BN_EPS)
    s3 = g3 / np.sqrt(v3 + BN_EPS)
    # conv1: y[co] = sum_ci w1[co,ci]*x[ci]; fold BN scale into co rows.
    # single e4m3 (the quantization noise washes out in the DHT bins);
    # layout [ci%128, half*128 + co].
    w1f = (w1[:, :, 0, 0] * s1[:, None]).T            # [ci=256, co=128]
    w1h = w1f.reshape(2, 128, 128).transpose(1, 0, 2)  # [ci128, half, co]
    w1p8 = np.ascontiguousarray(w1h.reshape(128, 256).astype(FP8))
    # bias enters the conv1 psum as a K=1 DoubleRow product (ones x bias);
    # hi/lo fp8 rows keep it exact to ~0.07%.  [1, 2*8*128]: [bhi x8|blo x8]
    bias1 = ((b1 - m1) * s1 + be1).astype(np.float32)
    b_hi = _q8(bias1)
    b_lo = bias1 - b_hi
    bias18 = np.ascontiguousarray(np.concatenate(
        [np.tile(b_hi, 8), np.tile(b_lo, 8)]).reshape(1, 2048).astype(FP8))

    # conv2/3: fp8 hi/lo split, layout [ci, hi(9*128) | lo(9*128) | zero(128)]
    def conv_w8(w, s, inv_in_scale):
        wf = (w * s[:, None, None, None]).transpose(2, 3, 1, 0)  # [ky,kx,ci,co]
        wf = wf.reshape(9, 128, 128).transpose(1, 0, 2)          # [ci,tap,co]
        wf = wf * inv_in_scale
        w_hi = _q8(wf)
        w_lo = wf - w_hi
        # [zero(128) | hi(1152) | lo(1152)]
        arr = np.zeros((128, 2432), np.float32)
        arr[:, 128:1280] = w_hi.reshape(128, 1152)
        arr[:, 1280:2432] = w_lo.reshape(128, 1152)
        return np.ascontiguousarray(arr.astype(FP8))

    w2p8 = conv_w8(w2, s2, 1.0 / SC1)
    bias2 = (((b2 - m2) * s2 + be2) * SC2).astype(np.float32).reshape(128, 1)
    w3p8 = conv_w8(w3, s3, 1.0 / SC2)
    bias3 = ((b3 - m3) * s3 + be3).astype(np.float32).reshape(128, 1)
    return w1p8, bias18, w2p8, bias2, w3p8, bias3


# ----------------------------------------------------------------------------
# walrus workaround: split multi-wait instructions (this build supports only
# one sync-wait per instruction)
# ----------------------------------------------------------------------------

def _split_multi_waits(nc, mybir, max_waits=1):
    cnt = 0
    for f in nc.m.functions:
        for bb in f.blocks:
            insts = list(bb.instructions)
            new = []
            changed = False
            for inst in insts:
                si = inst.sync_info
                if si is not None:
                    ow = list(si.on_wait)
                    if len(ow) > max_waits:
                        changed = True
                        head = ow[:-max_waits]
                        for i in range(0, len(head), max_waits):
                            nop = mybir.InstNoOp(name=f'waitsplit_{cnt}',
                                                 ins=[], outs=[])
                            cnt += 1
                            nop.engine = inst.engine
                            nop.sync_info = mybir.SyncInfo(
                                on_wait=head[i:i + max_waits], on_update=[])
                            new.append(nop)
                        si.on_wait = ow[-max_waits:]
                new.append(inst)
            if changed:
                bb.instructions = new
    return cnt


# ----------------------------------------------------------------------------
# bass program
# ----------------------------------------------------------------------------

_PROGRAM_CACHE = {}


def _build_program(split_waits=True):
    key = ('nc', split_waits)
    if key in _PROGRAM_CACHE:
        return _PROGRAM_CACHE[key]
    import concourse.bass as bass
    import concourse.mybir as mybir
    import concourse.tile as tile
    from concourse.ap import AP
    from contextlib import ExitStack

    plan = _dht_plan()
    ENTRIES = plan['entries']
    BAND_OFF = plan['band_off']
    OHMAX = plan['ohmax']
    OHTOT = BAND_OFF[-1]

    f32 = mybir.dt.float32
    bf16 = mybir.dt.bfloat16
    fp8 = mybir.dt.float8e4
    RELU = mybir.ActivationFunctionType.Relu
    COPY = mybir.ActivationFunctionType.Copy
    DR = mybir.MatmulPerfMode.DoubleRow

    nc = bass.Bass('TRN2', target_bir_lowering=False, debug=False)
    # x arrives host-permuted: col = chunk*228 + half*128 + p (halves of a
    # chunk 128 apart -- walrus dual-fp8 ldweights needs the row-group
    # stride to be a multiple of 128; the 28-byte gaps are zero)
    x_d = nc.dram_tensor('x', [128, NBLK * 228], fp8, kind='ExternalInput')
    w1_d = nc.dram_tensor('w1p8', [128, 256], fp8, kind='ExternalInput')
    b1_d = nc.dram_tensor('bias18', [1, 2048], fp8, kind='ExternalInput')
    w2_d = nc.dram_tensor('w2p8', [128, 2432], fp8, kind='ExternalInput')
    b2_d = nc.dram_tensor('bias2', [128, 1], f32, kind='ExternalInput')
    w3_d = nc.dram_tensor('w3p8', [128, 2432], fp8, kind='ExternalInput')
    b3_d = nc.dram_tensor('bias3', [128, 1], f32, kind='ExternalInput')
    oh_d = nc.dram_tensor('ohtab', [BP, OHTOT], fp8, kind='ExternalInput')
    out_d = nc.dram_tensor('out', [128, HW], f32, kind='ExternalOutput')

    def dr_ap(base_ap, offset, istride, icount, nstride, ncount, parts):
        """3-dim AP [parts, icount, ncount] for DoubleRow operands."""
        return AP(base_ap.tensor, base_ap.offset + offset,
                  [[base_ap.ap[0][0], parts],
                   [istride, icount], [nstride, ncount]])

    with tile.TileContext(nc) as tc, ExitStack() as st0:
        consts = st0.enter_context(tc.tile_pool(name='consts', bufs=1))
        h1t_pool = st0.enter_context(tc.tile_pool(name='h1t', bufs=1))
        pad_pool = st0.enter_context(tc.tile_pool(name='pads', bufs=1))
        outb_pool = st0.enter_context(tc.tile_pool(name='outb', bufs=3))
        oh_pool = st0.enter_context(tc.tile_pool(name='oh', bufs=3))

        w1_t = consts.tile([128, 256], fp8, tag='w1')
        w2_t = consts.tile([128, 2432], fp8, tag='w2')
        w3_t = consts.tile([128, 2432], fp8, tag='w3')
        b1_t = consts.tile([1, 2048], fp8, tag='b1')
        b2_t = consts.tile([128, 1], f32, tag='b2')
        b3_t = consts.tile([128, 1], f32, tag='b3')
        nc.scalar.dma_start(out=b1_t[:], in_=b1_d.ap())
        zero_t = consts.tile([128, 512], bf16, tag='zeros')
        nc.vector.memset(zero_t[:], 0.0)

        h1T = h1t_pool.tile([128, NBLK * 128], fp8, tag='h1T')

        oh_tiles = {}

        def issue_oh(b, eng):
            t = oh_pool.tile([128, OHMAX], fp8, tag='oh', name=f'oh_{b}')
            cols = BAND_OFF[b + 1] - BAND_OFF[b]
            eng.dma_start(out=t[:BP, :cols],
                          in_=oh_d.ap()[0:BP, BAND_OFF[b]:BAND_OFF[b + 1]])
            oh_tiles[b] = t

        # conv inputs as fp8 hi/lo pairs in ONE tile each ([lo | hi] halves
        # so all DoubleRow pair strides stay positive); h2_pad keeps the
        # bf16 conv2 output so lo = relu(psum+b) - hi is computable.
        HOFF = PADW * PADW
        dpad8 = pad_pool.tile([128, 2 * HOFF], fp8, tag='dpad8')
        hpad8 = pad_pool.tile([128, 2 * HOFF], fp8, tag='hpad8')
        h2_pad = pad_pool.tile([128, PADW * PADW], bf16, tag='h2_pad')

        def pad_border_memsets():
            # zero only the borders; the interior is fully overwritten.
            for pad_t in (dpad8, hpad8):
                pv = pad_t[:].rearrange('c (two a r) -> c two a r', two=2,
                                        a=PADW)
                nc.gpsimd.memset(pv[:, :, 0:1, :], 0.0)
                nc.gpsimd.memset(pv[:, :, PADW - 1:PADW, :], 0.0)
                nc.gpsimd.memset(pv[:, :, :, 0:1], 0.0)
                nc.gpsimd.memset(pv[:, :, :, PADW - 1:PADW], 0.0)

        with ExitStack() as stT:
            # -------------------------------------------- transposed conv1
            # x arrives pixel-blocked ([ci, chunk*100+p]) so each chunk is a
            # contiguous 100-col slab.  conv1 runs TRANSPOSED: h1T[p, co] =
            # x_chunk^T @ w1 (x stationary, weights moving), writing h1T
            # directly -- no PE transposes, no psum drains, no h1 buffer.
            # Bias rides in as a K=1 DoubleRow product (ones x [bhi|blo]).
            with ExitStack() as st1:
                xf_pool = st1.enter_context(tc.tile_pool(name='xf', bufs=2))
                xb_pool = st1.enter_context(tc.tile_pool(name='xb', bufs=1))
                ps1 = st1.enter_context(
                    tc.tile_pool(name='ps1', bufs=3, space='PSUM'))

                GW = 10 * 228              # 2280: one block-row of chunks
                xbig = xb_pool.tile([128, 8 * GW], fp8, tag='xbig')
                ones_t = consts.tile([1, 256], fp8, tag='ones')
                nc.vector.memset(ones_t[:], 1.0)

                xf_tiles = {}

                def x_lhsT(k):
                    """stationary x chunk [K=128, 2(half), 100(pix)]."""
                    g = k // 10
                    kk = k % 10
                    if g < 2:
                        xa = xf_tiles[g][:]
                        off = kk * 228
                    else:
                        xa = xbig[:]
                        off = (g - 2) * GW + kk * 228
                    return AP(xa.tensor, xa.offset + off,
                              [[xa.ap[0][0], 128], [128, 2], [1, 100]])

                def conv1_bank(b):
                    """8 chunks (4 for the tail bank) -> h1T fp8."""
                    k0 = b * 8
                    kc = min(8, NBLK - k0)
                    ps = ps1.tile([128, 1024], f32, tag='ps1',
                                  name=f'c1_{b}')
                    ba = b1_t[:]
                    oa = ones_t[:]
                    # bias init: one K=1 DoubleRow matmul per 512-col zero
                    # region (adds bhi + blo exactly)
                    for rg in range((kc + 3) // 4):
                        n = min(512, kc * 128 - rg * 512)
                        lhsT = AP(oa.tensor, oa.offset,
                                  [[oa.ap[0][0], 1], [128, 2], [1, 128]])
                        rhs = AP(ba.tensor, ba.offset + rg * 512,
                                 [[ba.ap[0][0], 1], [1024, 2], [1, n]])
                        nc.tensor.matmul(out=ps[:, rg * 512:rg * 512 + n],
                                         lhsT=lhsT, rhs=rhs, start=True,
                                         stop=False, perf_mode=DR,
                                         skip_group_check=True)
                    for kk in range(kc):
                        k = k0 + kk
                        rhs = dr_ap(w1_t[:], 0, 128, 2, 1, 128, 128)
                        nc.tensor.matmul(
                            out=ps[:BP, kk * 128:(kk + 1) * 128],
                            lhsT=x_lhsT(k), rhs=rhs, start=False,
                            stop=(kk in (3, kc - 1)), perf_mode=DR,
                            skip_group_check=True)
                    # epilogue: relu * SC1, psum -> h1T fp8
                    dst = h1T[:BP, k0 * 128:(k0 + kc) * 128]
                    src = ps[:BP, :kc * 128]
                    if b % 2 == 0:
                        nc.scalar.activation(out=dst, in_=src, func=RELU,
                                             scale=SC1)
                    else:
                        nc.vector.tensor_scalar(
                            out=dst, in0=src, scalar1=0.0, scalar2=SC1,
                            op0=mybir.AluOpType.max,
                            op1=mybir.AluOpType.mult)

                for g in range(2):
                    xf = xf_pool.tile([128, GW], fp8, tag='xf',
                                      name=f'xf_{g}')
                    xf_tiles[g] = xf
                    (nc.sync, nc.gpsimd)[g].dma_start(
                        out=xf[:], in_=x_d.ap()[:, g * GW:(g + 1) * GW])
                    if g == 0:
                        nc.sync.dma_start(out=w1_t[:], in_=w1_d.ap())
                    if g == 1:
                        # groups 2-9 as 2-group DMAs (balances HWDGE fixed
                        # cost against DMA-device hold time)
                        for gg in range(4):
                            (nc.sync, nc.gpsimd)[gg % 2].dma_start(
                                out=xbig[:, gg * 2 * GW:(gg + 1) * 2 * GW],
                                in_=x_d.ap()[:, (2 + 2 * gg) * GW:
                                             (4 + 2 * gg) * GW])
                # one-hot bands 0-2 + conv weights on the SYNC queue so
                # their HWDGE slots (and thus DMA-device FIFO positions)
                # fall BEHIND every x transfer
                issue_oh(0, nc.sync)
                issue_oh(1, nc.sync)
                issue_oh(2, nc.sync)
                nc.sync.dma_start(out=w2_t[:], in_=w2_d.ap())
                nc.sync.dma_start(out=w3_t[:], in_=w3_d.ap())
                for b in range(13):
                    conv1_bank(b)

            pad_border_memsets()
            nc.scalar.dma_start(out=b2_t[:], in_=b2_d.ap())
            nc.scalar.dma_start(out=b3_t[:], in_=b3_d.ap())

            psd = stT.enter_context(
                tc.tile_pool(name='psd', bufs=4, space='PSUM'))
            psc = stT.enter_context(
                tc.tile_pool(name='psc', bufs=2, space='PSUM'))

            # -------------------------------------------------- DHT + convs
            def zero_bank(b, bank):
                # initialize the accumulator: all DHT matmuls use start=False
                # and accumulate onto zeroed PSUM.
                if b < 1:
                    nc.tensor.matmul(out=bank[:, :BAND * R],
                                     lhsT=zero_t[:1, :128],
                                     rhs=zero_t[:1, :BAND * R], start=True,
                                     stop=False, skip_group_check=True)
                elif b == 1:
                    nc.scalar.activation(out=bank[:, :BAND * R],
                                         in_=zero_t[:, :BAND * R],
                                         func=COPY)
                else:
                    nc.vector.memset(bank[:, :BAND * R], 0.0)

            def dht_band(b, bank):
                oh = oh_tiles[b]
                off0 = BAND_OFF[b]
                for s in range(BAND):
                    a = b * BAND + s
                    for ent in ENTRIES[a]:
                        if ent[0] == 'p':
                            _, k1, k2, lo_u, win_u, coloff = ent
                            lhsT = dr_ap(h1T[:], k1 * 128,
                                         (k2 - k1) * 128, 2, 1, 128, BP)
                            rhs = dr_ap(oh[:], coloff - off0,
                                        win_u, 2, 1, win_u, BP)
                            nc.tensor.matmul(
                                out=bank[:, s * R + lo_u:
                                         s * R + lo_u + win_u],
                                lhsT=lhsT, rhs=rhs, start=False, stop=False,
                                skip_group_check=True, perf_mode=DR)
                        else:
                            _, k, lo_s, win_s, coloff = ent
                            c0 = coloff - off0
                            nc.tensor.matmul(
                                out=bank[:, s * R + lo_s:
                                         s * R + lo_s + win_s],
                                lhsT=h1T[:BP, k * 128:(k + 1) * 128],
                                rhs=oh[:BP, c0:c0 + win_s],
                                start=False, stop=False,
                                skip_group_check=True)

            def pad_views(pad_t, a0, na):
                """(lo, hi) interior views [c, na, R] of a hi/lo pad tile."""
                pv = pad_t[:].rearrange('c (two a r) -> c two a r', two=2,
                                        a=PADW)
                lo = pv[:, 0:1, a0 + 1:a0 + 1 + na, 1:1 + R].squeeze(1)
                hi = pv[:, 1:2, a0 + 1:a0 + 1 + na, 1:1 + R].squeeze(1)
                return lo, hi

            def drain_band(b, bank):
                # psum -> fp8 hi (ACT) + fp8 lo residual (DVE subtract)
                a0 = b * BAND
                pv = bank[:, :BAND * R].rearrange('p (a r) -> p a r', a=BAND)
                lo_v, hi_v = pad_views(dpad8, a0, BAND)
                nc.scalar.activation(out=hi_v, in_=pv[:], func=COPY)
                nc.vector.tensor_tensor(out=lo_v, in0=pv[:], in1=hi_v,
                                        op=mybir.AluOpType.subtract)

            # conv psum layout: per band one [128, 1024] tile = 2 zero
            # regions; sub-band s2 (2 angles) accumulates over the FLATTENED
            # padded rows at cols [s2*512, s2*512+202) (junk at row-boundary
            # cols, skipped by the epilogue's PADW-strided reads).
            LSUB = PADW + R    # 202

            def emit14(ps, s2, pad_t, w_t, a0, lsub=None):
                """3-product fp8 conv: 14 DoubleRow passes accumulating
                w_hi*x_hi + w_hi*x_lo + w_lo*x_hi over 9 taps."""
                def wc(g, t):
                    return 128 + g * 1152 + t * 128

                def xc(h, t):
                    return (h * HOFF + (t // 3) * PADW + (t % 3)
                            + a0 * PADW)

                pairs = []
                for t in (0, 2, 4, 6):
                    pairs.append(((wc(0, t), xc(1, t)),
                                  (wc(0, t + 1), xc(1, t + 1))))
                for t in (0, 2, 4, 6):
                    pairs.append(((wc(0, t), xc(0, t)),
                                  (wc(0, t + 1), xc(0, t + 1))))
                for t in (0, 2, 4, 6):
                    pairs.append(((wc(1, t), xc(1, t)),
                                  (wc(1, t + 1), xc(1, t + 1))))
                pairs.append(((wc(0, 8), xc(0, 8)), (wc(1, 8), xc(1, 8))))
                # odd 27th product pairs with the zero-weight block (w col
                # 0); the dummy rhs member re-reads tap-0's valid window
                pairs.append(((0, xc(1, 0)), (wc(0, 8), xc(1, 8))))
                wa = w_t[:]
                xa = pad_t[:]
                if lsub is None:
                    lsub = LSUB
                out_v = ps[:, s2 * 512:s2 * 512 + lsub]
                for i, ((w1c, x1c), (w2c, x2c)) in enumerate(pairs):
                    lhsT = AP(wa.tensor, wa.offset + w1c,
                              [[wa.ap[0][0], 128], [w2c - w1c, 2], [1, 128]])
                    rhs = AP(xa.tensor, xa.offset + x1c,
                             [[xa.ap[0][0], 128], [x2c - x1c, 2], [1, lsub]])
                    nc.tensor.matmul(out=out_v, lhsT=lhsT, rhs=rhs,
                                     start=(i == 0),
                                     stop=(i == len(pairs) - 1),
                                     perf_mode=DR)

            def conv_psum_view(ps, ar, sub=0):
                """[p, (sub, al), R] view of the padded conv psum."""
                pa = ps[:]
                if ar == BAND:
                    return AP(pa.tensor, pa.offset,
                              [[pa.ap[0][0], 128], [512, 2], [PADW, 2],
                               [1, R]])
                if ar == 1:
                    return AP(pa.tensor, pa.offset + sub * 512,
                              [[pa.ap[0][0], 128], [1, R]])
                return AP(pa.tensor, pa.offset,
                          [[pa.ap[0][0], 128], [PADW, ar], [1, R]])

            def conv2_band(c):
                a0 = c * BAND
                ps = psc.tile([128, 1024], f32, tag='conv')
                for s2 in range(2):
                    emit14(ps, s2, dpad8, w2_t, a0 + s2 * 2)
                pv = conv_psum_view(ps, BAND)
                hv2 = h2_pad[:].rearrange('c (a r) -> c a r', a=PADW)
                h2v = hv2[:, a0 + 1:a0 + 1 + BAND, 1:1 + R]
                h2v4 = h2v.rearrange('c (x y) r -> c x y r', x=2)
                nc.scalar.activation(out=h2v4, in_=pv, func=RELU,
                                     bias=b2_t[:, :1], scale=SC2)
                lo_v, hi_v = pad_views(hpad8, a0, BAND)
                nc.scalar.activation(out=hi_v, in_=h2v, func=COPY)
                nc.vector.tensor_tensor(out=lo_v, in0=h2v, in1=hi_v,
                                        op=mybir.AluOpType.subtract)

            def conv3_band(c, ar=BAND, s0=0, dve_epi=False, ps=None, sub=0):
                a0 = c * BAND + s0
                if ps is None:
                    ps = psc.tile([128, 1024], f32, tag='conv')
                if ar == 1:
                    emit14(ps, sub, hpad8, w3_t, a0, lsub=R)
                else:
                    for s2 in range(ar // 2):
                        emit14(ps, s2, hpad8, w3_t, a0 + s2 * 2)
                pv = conv_psum_view(ps, ar, sub)
                ob = outb_pool.tile([128, ar * R], f32, tag=f'outb{ar}')
                if ar == BAND:
                    ov = ob[:].rearrange('p (x y r) -> p x y r', x=2, y=2)
                elif ar == 1:
                    ov = ob[:]
                else:
                    ov = ob[:].rearrange('p (a r) -> p a r', a=ar)
                if dve_epi:
                    nc.vector.tensor_scalar(
                        out=ov, in0=pv, scalar1=b3_t[:, :1],
                        scalar2=0.0, op0=mybir.AluOpType.add,
                        op1=mybir.AluOpType.max)
                else:
                    nc.scalar.activation(out=ov, in_=pv, func=RELU,
                                         bias=b3_t[:, :1], scale=1.0)
                nc.sync.dma_start(out=out_d.ap()[:, a0 * R:(a0 + ar) * R],
                                  in_=ob[:])

            banks = {0: psd.tile([128, 512], f32, tag='band',
                                 name='bank_0')}
            zero_bank(0, banks[0])
            for b in range(NBAND):
                # stream the upcoming one-hot bands behind the PE
                if b + 3 < NBAND:
                    issue_oh(b + 3, (nc.sync, nc.gpsimd, nc.scalar)[b % 3])
                if b + 1 < NBAND:
                    banks[b + 1] = psd.tile([128, 512], f32, tag='band',
                                            name=f'bank_{b + 1}')
                    zero_bank(b + 1, banks[b + 1])
                dht_band(b, banks[b])
                drain_band(b, banks[b])
                del banks[b]
                del oh_tiles[b]
                if b >= 2:
                    conv2_band(b - 2)
                if b >= 4:
                    conv3_band(b - 4)
            for c in (NBAND - 2, NBAND - 1):
                conv2_band(c)
            for c in range(NBAND - 4, NBAND - 1):
                conv3_band(c)
            # split the last band so its epilogue/DMA pipeline with the
            # later pieces' matmuls instead of trailing the whole kernel
            conv3_band(NBAND - 1, ar=2, s0=0)
            ps_tail = psc.tile([128, 1024], f32, tag='conv')
            conv3_band(NBAND - 1, ar=1, s0=2, ps=ps_tail, sub=0)
            conv3_band(NBAND - 1, ar=1, s0=3, dve_epi=True, ps=ps_tail,
                       sub=1)

    if split_waits:
        _split_multi_waits(nc, mybir)
    _PROGRAM_CACHE[key] = nc
    return nc


# ----------------------------------------------------------------------------
# entry point
# ----------------------------------------------------------------------------

def make_in_maps(inputs):
    plan = _dht_plan()
    x = np.asarray(inputs['x'], np.float32)
    w1p8, bias18, w2p8, bias2, w3p8, bias3 = _prep_weights(
        *[np.asarray(inputs[k], np.float32) for k in
          ('w1', 'b1', 'g1', 'be1', 'm1', 'v1',
           'w2', 'b2', 'g2', 'be2', 'm2', 'v2',
           'w3', 'b3', 'g3', 'be3', 'm3', 'v3')])
    common = dict(w1p8=w1p8, bias18=bias18, w2p8=w2p8, bias2=bias2,
                  w3p8=w3p8, bias3=bias3, ohtab=plan['ohtab'])
    # x host-permuted: [ci%128, chunk*228 + (ci//128)*128 + p] with
    # p = dy*10+dx, chunk = gy*10+gx; 28-byte zero gaps per chunk
    xb = (x.reshape(N, 2, 128, YBLK, BH, XBLK, BW)
          .transpose(0, 2, 3, 5, 1, 4, 6)      # n, ci128, gy, gx, half, dy, dx
          .reshape(N, 128, NBLK, 2, BP))
    xp = np.zeros((N, 128, NBLK, 228), FP8)
    xp[:, :, :, 0:100] = xb[:, :, :, 0].astype(FP8)
    xp[:, :, :, 128:228] = xb[:, :, :, 1].astype(FP8)
    xp = xp.reshape(N, 128, NBLK * 228)
    return [
        {'x': np.ascontiguousarray(xp[n]), **common}
        for n in range(N)
    ]


def run(inputs, trace=False):
    from concourse.bass_utils import run_bass_kernel_spmd

    nc = _build_program()
    in_maps = make_in_maps(inputs)
    res = run_bass_kernel_spmd(nc, in_maps, core_ids=list(range(N)),
                               trace=trace)
    out = np.stack([res.results[n]['out'].reshape(CMID, H, W)
                    for n in range(N)], axis=0)
    return out.astype(np.float32), res


def kernel(**inputs):
    out, _ = run(inputs, trace=False)
    return out


# revision 63
# speedup vs baseline: 1.5473x; 1.0151x over previous
"""Trainium2 Bass kernel for nn_DHT_Layer (conv1x1+BN+ReLU -> Deep Hough
Transform -> two 3x3 conv+BN+ReLU layers).

Sharding: data-parallel over batch. 8 images / 8 cores -> one image per core,
no collectives; full inputs in, full output out. Inside each core:
  conv1   : 1x1 conv in fp8e4 DoubleRow mode (K=256 per pass), weights split
            hi+lo e4m3 for accuracy -> 2 passes at 0.5 cyc/col (10k cyc
            total vs 20k bf16).  x is quantized e4m3 on host; the resulting
            per-element noise averages out in the DHT's ~100-pixel
            positive-sum bins (~0.4% end-to-end).  BN+ReLU epilogues write
            bf16 h1 in a pixel-blocked layout; PE transposes (bf16) scatter
            it to h1T with the PSUM->SBUF drains converting to fp8e4.
  DHT     : out[c,a,r] = sum_p h[c,p] * (idx[a,p]==r) as fp8 one-hot
            matmuls.  Pixels are chunked 10x10 (100 chunks of 100 pixels);
            per angle, chunks are PAIRED along the direction that minimizes
            the rho-window union (x/y/diag/antidiag) and each pair runs as
            one DoubleRow matmul (two K=100 slices, 0.5 cyc/col over the
            union window); leftovers run as plain fp8 matmuls.  Total
            ~28.8k cyc-equivalents vs 90.7k bf16 baseline.  The fp8
            one-hots are precomputed on host (geometry-only) and streamed
            per 4-angle band via DMA (~14 MB, ~31 us, fully overlapped).
  conv2/3 : 3x3 convs as 9 shifted bf16 matmuls over a zero-padded
            [c, 102*102] layout, BN+ReLU folded into the epilogue;
            interleaved into the DHT band loop (conv2 lags 2 bands, conv3
            lags 4) so PE never waits on an ACT drain it just requested.

The local walrus build only supports ONE sync-wait per instruction, so a
post-pass splits multi-wait instructions into single-wait NoOp carriers.
"""

import functools
import math

import ml_dtypes
import numpy as np

N = 8          # batch / cores
CIN = 256
CMID = 128
H = W = 100
HW = H * W
A = 100        # angles
R = 100        # rho bins
BH, BW = 10, 10
YBLK = 10
XBLK = 10
NBLK = YBLK * XBLK             # 100 chunks
BP = BH * BW                   # 100 pixels per chunk (contraction dim)
SROWS = 5      # conv1 slice height (rows per psum chunk)
BAND = 4       # angles per PSUM bank / conv2 row band
NBAND = A // BAND
PADW = W + 2   # 102 padded spatial for 3x3 convs
BN_EPS = 1e-5
BF16 = ml_dtypes.bfloat16
FP8 = ml_dtypes.float8_e4m3
# power-of-two activation scales keep fp8 in range (DHT sums reach ~920,
# conv2+BN outputs ~1700); exact in fp8, inverses folded into w2/w3
SC1 = 2.0 ** -4     # h1T / DHT domain
SC2 = 2.0 ** -5     # h2 / conv3-input domain


# ----------------------------------------------------------------------------
# host-side precomputation (shapes are fixed -> cache)
# ----------------------------------------------------------------------------

def _hough_idx():
    irho = int(math.sqrt(H * H + W * W) + 1) / float(R)
    theta = np.arange(A) * (math.pi / A)
    tab_cos = np.cos(theta) / irho
    tab_sin = np.sin(theta) / irho
    yy, xx = np.meshgrid(np.arange(H) - H // 2, np.arange(W) - W // 2,
                         indexing='ij')
    r = np.round(xx[None, :, :] * tab_cos[:, None, None]
                 + yy[None, :, :] * tab_sin[:, None, None])
    return np.clip(r + R // 2, 0, R - 1).astype(np.int32)  # [A, H, W]


@functools.lru_cache(maxsize=1)
def _dht_plan():
    """Per-angle DoubleRow pairing plan + host-built fp8 one-hot table.

    Returns dict with:
      entries[a]: list of ('p', k1, k2, lo_u, win_u, coloff) and
                  ('s', k, lo, win, coloff); coloff is absolute into ohtab.
      band_off[b]: first ohtab column of band b (b in 0..NBAND, sentinel).
      ohtab: [BP, TOT] fp8 one-hot table.
      ohmax: max columns of any band.
    """
    idx = _hough_idx()
    lo = np.zeros((A, NBLK), np.int64)
    hi = np.zeros((A, NBLK), np.int64)
    # pix[k, p] = (y, x) of partition p in chunk k
    sub_idx = np.zeros((A, NBLK, BP), np.int64)
    for gy in range(YBLK):
        for gx in range(XBLK):
            k = gy * XBLK + gx
            sub = idx[:, gy * BH:(gy + 1) * BH,
                      gx * BW:(gx + 1) * BW].reshape(A, BP)
            sub_idx[:, k] = sub
            lo[:, k] = sub.min(axis=1)
            hi[:, k] = sub.max(axis=1)
    win = hi - lo + 1

    def pairing(a):
        """Greedy min-union matching over lo-sorted neighbors; leftovers
        pair with the h1T zero block (chunk id NBLK) at 0.5x their window."""
        order = np.argsort(lo[a], kind='stable')
        cand = []
        for i in range(NBLK):
            for j in range(i + 1, min(i + 9, NBLK)):
                k1, k2 = int(order[i]), int(order[j])
                u = (max(hi[a, k1], hi[a, k2])
                     - min(lo[a, k1], lo[a, k2]) + 1)
                cand.append((u, k1, k2))
        cand.sort()
        used = np.zeros(NBLK, bool)
        pairs = []
        for u, k1, k2 in cand:
            if used[k1] or used[k2]:
                continue
            used[k1] = used[k2] = True
            pairs.append((k1, k2))
            if len(pairs) == NBLK // 2:
                break
        singles = [int(k) for k in np.where(~used)[0]]
        return pairs, singles

    entries = []
    band_off = [0]
    ohcols = []        # list of np arrays [BP] per column
    for b in range(NBAND):
        for s in range(BAND):
            a = b * BAND + s
            pairs, singles = pairing(a)
            # leftovers pair with the zero block (chunk NBLK); their second
            # one-hot member duplicates the first (zero weights kill it)
            jobs = ([(k1, k2, k1, k2) for k1, k2 in pairs]
                    + [(k, NBLK, k, k) for k in singles])
            ents = []
            for k1, k2, m1, m2 in jobs:
                lo_u = int(min(lo[a, m1], lo[a, m2]))
                win_u = int(max(hi[a, m1], hi[a, m2])) - lo_u + 1
                coloff = len(ohcols)
                for k in (m1, m2):
                    rel = sub_idx[a, k] - lo_u          # [BP]
                    oh = np.zeros((win_u, BP), np.float32)
                    oh[rel, np.arange(BP)] = 1.0
                    for j in range(win_u):
                        ohcols.append(oh[j])
                ents.append(('p', k1, k2, lo_u, win_u, coloff))
            entries.append(ents)
        band_off.append(len(ohcols))
    ohtab = np.ascontiguousarray(
        np.stack(ohcols, axis=1).astype(FP8))       # [BP, TOT]
    ohmax = max(band_off[i + 1] - band_off[i] for i in range(NBAND))
    return dict(entries=entries, band_off=band_off, ohtab=ohtab,
                ohmax=ohmax)


def _q8(x):
    return x.astype(FP8).astype(np.float32)


def _prep_weights(w1, b1, g1, be1, m1, v1, w2, b2, g2, be2, m2, v2,
                  w3, b3, g3, be3, m3, v3):
    s1 = g1 / np.sqrt(v1 + BN_EPS)
    s2 = g2 / np.sqrt(v2 + BN_EPS)
    s3 = g3 / np.sqrt(v3 + BN_EPS)
    # conv1: y[co] = sum_ci w1[co,ci]*x[ci]; fold BN scale into co rows.
    # single e4m3 (the quantization noise washes out in the DHT bins);
    # layout [ci%128, half*128 + co].
    w1f = (w1[:, :, 0, 0] * s1[:, None]).T            # [ci=256, co=128]
    w1h = w1f.reshape(2, 128, 128).transpose(1, 0, 2)  # [ci128, half, co]
    w1p8 = np.ascontiguousarray(w1h.reshape(128, 256).astype(FP8))
    # bias enters the conv1 psum as a K=1 DoubleRow product (ones x bias);
    # hi/lo fp8 rows keep it exact to ~0.07%.  [1, 2*8*128]: [bhi x8|blo x8]
    bias1 = ((b1 - m1) * s1 + be1).astype(np.float32)
    b_hi = _q8(bias1)
    b_lo = bias1 - b_hi
    bias18 = np.ascontiguousarray(np.concatenate(
        [np.tile(b_hi, 8), np.tile(b_lo, 8)]).reshape(1, 2048).astype(FP8))

    # conv2/3: fp8 hi/lo split, layout [ci, hi(9*128) | lo(9*128) | zero(128)]
    def conv_w8(w, s, inv_in_scale):
        wf = (w * s[:, None, None, None]).transpose(2, 3, 1, 0)  # [ky,kx,ci,co]
        wf = wf.reshape(9, 128, 128).transpose(1, 0, 2)          # [ci,tap,co]
        wf = wf * inv_in_scale
        w_hi = _q8(wf)
        w_lo = wf - w_hi
        # [zero(128) | hi(1152) | lo(1152)]
        arr = np.zeros((128, 2432), np.float32)
        arr[:, 128:1280] = w_hi.reshape(128, 1152)
        arr[:, 1280:2432] = w_lo.reshape(128, 1152)
        return np.ascontiguousarray(arr.astype(FP8))

    w2p8 = conv_w8(w2, s2, 1.0 / SC1)
    bias2 = (((b2 - m2) * s2 + be2) * SC2).astype(np.float32).reshape(128, 1)
    w3p8 = conv_w8(w3, s3, 1.0 / SC2)
    bias3 = ((b3 - m3) * s3 + be3).astype(np.float32).reshape(128, 1)
    return w1p8, bias18, w2p8, bias2, w3p8, bias3


# ----------------------------------------------------------------------------
# walrus workaround: split multi-wait instructions (this build supports only
# one sync-wait per instruction)
# ----------------------------------------------------------------------------

def _split_multi_waits(nc, mybir, max_waits=1):
    cnt = 0
    for f in nc.m.functions:
        for bb in f.blocks:
            insts = list(bb.instructions)
            new = []
            changed = False
            for inst in insts:
                si = inst.sync_info
                if si is not None:
                    ow = list(si.on_wait)
                    if len(ow) > max_waits:
                        changed = True
                        head = ow[:-max_waits]
                        for i in range(0, len(head), max_waits):
                            nop = mybir.InstNoOp(name=f'waitsplit_{cnt}',
                                                 ins=[], outs=[])
                            cnt += 1
                            nop.engine = inst.engine
                            nop.sync_info = mybir.SyncInfo(
                                on_wait=head[i:i + max_waits], on_update=[])
                            new.append(nop)
                        si.on_wait = ow[-max_waits:]
                new.append(inst)
            if changed:
                bb.instructions = new
    return cnt


# ----------------------------------------------------------------------------
# bass program
# ----------------------------------------------------------------------------

_PROGRAM_CACHE = {}


def _build_program(split_waits=True):
    key = ('nc', split_waits)
    if key in _PROGRAM_CACHE:
        return _PROGRAM_CACHE[key]
    import concourse.bass as bass
    import concourse.mybir as mybir
    import concourse.tile as tile
    from concourse.ap import AP
    from contextlib import ExitStack

    plan = _dht_plan()
    ENTRIES = plan['entries']
    BAND_OFF = plan['band_off']
    OHMAX = plan['ohmax']
    OHTOT = BAND_OFF[-1]

    f32 = mybir.dt.float32
    bf16 = mybir.dt.bfloat16
    fp8 = mybir.dt.float8e4
    RELU = mybir.ActivationFunctionType.Relu
    COPY = mybir.ActivationFunctionType.Copy
    DR = mybir.MatmulPerfMode.DoubleRow

    nc = bass.Bass('TRN2', target_bir_lowering=False, debug=False)
    # x arrives host-permuted: col = chunk*228 + half*128 + p (halves of a
    # chunk 128 apart -- walrus dual-fp8 ldweights needs the row-group
    # stride to be a multiple of 128; the 28-byte gaps are zero)
    x_d = nc.dram_tensor('x', [128, NBLK * 228], fp8, kind='ExternalInput')
    w1_d = nc.dram_tensor('w1p8', [128, 256], fp8, kind='ExternalInput')
    b1_d = nc.dram_tensor('bias18', [1, 2048], fp8, kind='ExternalInput')
    w2_d = nc.dram_tensor('w2p8', [128, 2432], fp8, kind='ExternalInput')
    b2_d = nc.dram_tensor('bias2', [128, 1], f32, kind='ExternalInput')
    w3_d = nc.dram_tensor('w3p8', [128, 2432], fp8, kind='ExternalInput')
    b3_d = nc.dram_tensor('bias3', [128, 1], f32, kind='ExternalInput')
    oh_d = nc.dram_tensor('ohtab', [BP, OHTOT], fp8, kind='ExternalInput')
    out_d = nc.dram_tensor('out', [128, HW], f32, kind='ExternalOutput')

    def dr_ap(base_ap, offset, istride, icount, nstride, ncount, parts):
        """3-dim AP [parts, icount, ncount] for DoubleRow operands."""
        return AP(base_ap.tensor, base_ap.offset + offset,
                  [[base_ap.ap[0][0], parts],
                   [istride, icount], [nstride, ncount]])

    with tile.TileContext(nc) as tc, ExitStack() as st0:
        consts = st0.enter_context(tc.tile_pool(name='consts', bufs=1))
        h1t_pool = st0.enter_context(tc.tile_pool(name='h1t', bufs=1))
        pad_pool = st0.enter_context(tc.tile_pool(name='pads', bufs=1))
        outb_pool = st0.enter_context(tc.tile_pool(name='outb', bufs=3))
        oh_pool = st0.enter_context(tc.tile_pool(name='oh', bufs=3))

        w1_t = consts.tile([128, 256], fp8, tag='w1')
        w2_t = consts.tile([128, 2432], fp8, tag='w2')
        w3_t = consts.tile([128, 2432], fp8, tag='w3')
        b1_t = consts.tile([1, 2048], fp8, tag='b1')
        b2_t = consts.tile([128, 1], f32, tag='b2')
        b3_t = consts.tile([128, 1], f32, tag='b3')
        nc.scalar.dma_start(out=b1_t[:], in_=b1_d.ap())
        zero_t = consts.tile([128, 512], bf16, tag='zeros')
        nc.vector.memset(zero_t[:], 0.0)

        # chunk slot NBLK is an all-zero block: leftover DHT singles pair
        # against it so every DHT matmul runs in DoubleRow mode
        h1T = h1t_pool.tile([128, (NBLK + 1) * 128], fp8, tag='h1T')
        nc.vector.memset(h1T[:, NBLK * 128:(NBLK + 1) * 128], 0.0)

        oh_tiles = {}

        def issue_oh(b, eng):
            t = oh_pool.tile([128, OHMAX], fp8, tag='oh', name=f'oh_{b}')
            cols = BAND_OFF[b + 1] - BAND_OFF[b]
            eng.dma_start(out=t[:BP, :cols],
                          in_=oh_d.ap()[0:BP, BAND_OFF[b]:BAND_OFF[b + 1]])
            oh_tiles[b] = t

        # conv inputs as fp8 hi/lo pairs in ONE tile each ([lo | hi] halves
        # so all DoubleRow pair strides stay positive); h2_pad keeps the
        # bf16 conv2 output so lo = relu(psum+b) - hi is computable.
        HOFF = PADW * PADW
        dpad8 = pad_pool.tile([128, 2 * HOFF], fp8, tag='dpad8')
        hpad8 = pad_pool.tile([128, 2 * HOFF], fp8, tag='hpad8')
        h2_pad = pad_pool.tile([128, PADW * PADW], bf16, tag='h2_pad')

        def pad_border_memsets():
            # zero only the borders; the interior is fully overwritten.
            for pad_t in (dpad8, hpad8):
                pv = pad_t[:].rearrange('c (two a r) -> c two a r', two=2,
                                        a=PADW)
                nc.gpsimd.memset(pv[:, :, 0:1, :], 0.0)
                nc.gpsimd.memset(pv[:, :, PADW - 1:PADW, :], 0.0)
                nc.gpsimd.memset(pv[:, :, :, 0:1], 0.0)
                nc.gpsimd.memset(pv[:, :, :, PADW - 1:PADW], 0.0)

        with ExitStack() as stT:
            # -------------------------------------------- transposed conv1
            # x arrives pixel-blocked ([ci, chunk*100+p]) so each chunk is a
            # contiguous 100-col slab.  conv1 runs TRANSPOSED: h1T[p, co] =
            # x_chunk^T @ w1 (x stationary, weights moving), writing h1T
            # directly -- no PE transposes, no psum drains, no h1 buffer.
            # Bias rides in as a K=1 DoubleRow product (ones x [bhi|blo]).
            with ExitStack() as st1:
                xf_pool = st1.enter_context(tc.tile_pool(name='xf', bufs=2))
                xb_pool = st1.enter_context(tc.tile_pool(name='xb', bufs=1))
                ps1 = st1.enter_context(
                    tc.tile_pool(name='ps1', bufs=3, space='PSUM'))

                GW = 10 * 228              # 2280: one block-row of chunks
                xbig = xb_pool.tile([128, 8 * GW], fp8, tag='xbig')
                ones_t = consts.tile([1, 256], fp8, tag='ones')
                nc.vector.memset(ones_t[:], 1.0)

                xf_tiles = {}

                def x_lhsT(k):
                    """stationary x chunk [K=128, 2(half), 100(pix)]."""
                    g = k // 10
                    kk = k % 10
                    if g < 2:
                        xa = xf_tiles[g][:]
                        off = kk * 228
                    else:
                        xa = xbig[:]
                        off = (g - 2) * GW + kk * 228
                    return AP(xa.tensor, xa.offset + off,
                              [[xa.ap[0][0], 128], [128, 2], [1, 100]])

                def conv1_bank(b):
                    """8 chunks (4 for the tail bank) -> h1T fp8."""
                    k0 = b * 8
                    kc = min(8, NBLK - k0)
                    ps = ps1.tile([128, 1024], f32, tag='ps1',
                                  name=f'c1_{b}')
                    ba = b1_t[:]
                    oa = ones_t[:]
                    # bias init: one K=1 DoubleRow matmul per 512-col zero
                    # region (adds bhi + blo exactly)
                    for rg in range((kc + 3) // 4):
                        n = min(512, kc * 128 - rg * 512)
                        lhsT = AP(oa.tensor, oa.offset,
                                  [[oa.ap[0][0], 1], [128, 2], [1, 128]])
                        rhs = AP(ba.tensor, ba.offset + rg * 512,
                                 [[ba.ap[0][0], 1], [1024, 2], [1, n]])
                        nc.tensor.matmul(out=ps[:, rg * 512:rg * 512 + n],
                                         lhsT=lhsT, rhs=rhs, start=True,
                                         stop=False, perf_mode=DR,
                                         skip_group_check=True)
                    for kk in range(kc):
                        k = k0 + kk
                        rhs = dr_ap(w1_t[:], 0, 128, 2, 1, 128, 128)
                        nc.tensor.matmul(
                            out=ps[:BP, kk * 128:(kk + 1) * 128],
                            lhsT=x_lhsT(k), rhs=rhs, start=False,
                            stop=(kk in (3, kc - 1)), perf_mode=DR,
                            skip_group_check=True)
                    # epilogue: relu * SC1, psum -> h1T fp8
                    dst = h1T[:BP, k0 * 128:(k0 + kc) * 128]
                    src = ps[:BP, :kc * 128]
                    if b % 2 == 0:
                        nc.scalar.activation(out=dst, in_=src, func=RELU,
                                             scale=SC1)
                    else:
                        nc.vector.tensor_scalar(
                            out=dst, in0=src, scalar1=0.0, scalar2=SC1,
                            op0=mybir.AluOpType.max,
                            op1=mybir.AluOpType.mult)

                for g in range(2):
                    xf = xf_pool.tile([128, GW], fp8, tag='xf',
                                      name=f'xf_{g}')
                    xf_tiles[g] = xf
                    (nc.sync, nc.gpsimd)[g].dma_start(
                        out=xf[:], in_=x_d.ap()[:, g * GW:(g + 1) * GW])
                    if g == 0:
                        nc.sync.dma_start(out=w1_t[:], in_=w1_d.ap())
                    if g == 1:
                        # groups 2-9 as 2-group DMAs (balances HWDGE fixed
                        # cost against DMA-device hold time)
                        for gg in range(4):
                            (nc.sync, nc.gpsimd)[gg % 2].dma_start(
                                out=xbig[:, gg * 2 * GW:(gg + 1) * 2 * GW],
                                in_=x_d.ap()[:, (2 + 2 * gg) * GW:
                                             (4 + 2 * gg) * GW])
                # one-hot bands 0-2 + conv weights on the SYNC queue so
                # their HWDGE slots (and thus DMA-device FIFO positions)
                # fall BEHIND every x transfer
                issue_oh(0, nc.sync)
                issue_oh(1, nc.sync)
                issue_oh(2, nc.sync)
                nc.sync.dma_start(out=w2_t[:], in_=w2_d.ap())
                nc.sync.dma_start(out=w3_t[:], in_=w3_d.ap())
                for b in range(13):
                    conv1_bank(b)

            pad_border_memsets()
            nc.scalar.dma_start(out=b2_t[:], in_=b2_d.ap())
            nc.scalar.dma_start(out=b3_t[:], in_=b3_d.ap())

            psd = stT.enter_context(
                tc.tile_pool(name='psd', bufs=4, space='PSUM'))
            psc = stT.enter_context(
                tc.tile_pool(name='psc', bufs=2, space='PSUM'))

            # -------------------------------------------------- DHT + convs
            def zero_bank(b, bank):
                # initialize the accumulator: all DHT matmuls use start=False
                # and accumulate onto zeroed PSUM.
                if b < 1:
                    nc.tensor.matmul(out=bank[:, :BAND * R],
                                     lhsT=zero_t[:1, :128],
                                     rhs=zero_t[:1, :BAND * R], start=True,
                                     stop=False, skip_group_check=True)
                elif b == 1:
                    nc.scalar.activation(out=bank[:, :BAND * R],
                                         in_=zero_t[:, :BAND * R],
                                         func=COPY)
                else:
                    nc.vector.memset(bank[:, :BAND * R], 0.0)

            def dht_band(b, bank):
                oh = oh_tiles[b]
                off0 = BAND_OFF[b]
                for s in range(BAND):
                    a = b * BAND + s
                    for ent in ENTRIES[a]:
                        if ent[0] == 'p':
                            _, k1, k2, lo_u, win_u, coloff = ent
                            lhsT = dr_ap(h1T[:], k1 * 128,
                                         (k2 - k1) * 128, 2, 1, 128, BP)
                            rhs = dr_ap(oh[:], coloff - off0,
                                        win_u, 2, 1, win_u, BP)
                            nc.tensor.matmul(
                                out=bank[:, s * R + lo_u:
                                         s * R + lo_u + win_u],
                                lhsT=lhsT, rhs=rhs, start=False, stop=False,
                                skip_group_check=True, perf_mode=DR)
                        else:
                            _, k, lo_s, win_s, coloff = ent
                            c0 = coloff - off0
                            nc.tensor.matmul(
                                out=bank[:, s * R + lo_s:
                                         s * R + lo_s + win_s],
                                lhsT=h1T[:BP, k * 128:(k + 1) * 128],
                                rhs=oh[:BP, c0:c0 + win_s],
                                start=False, stop=False,
                                skip_group_check=True)

            def pad_views(pad_t, a0, na):
                """(lo, hi) interior views [c, na, R] of a hi/lo pad tile."""
                pv = pad_t[:].rearrange('c (two a r) -> c two a r', two=2,
                                        a=PADW)
                lo = pv[:, 0:1, a0 + 1:a0 + 1 + na, 1:1 + R].squeeze(1)
                hi = pv[:, 1:2, a0 + 1:a0 + 1 + na, 1:1 + R].squeeze(1)
                return lo, hi

            def drain_band(b, bank):
                # psum -> fp8 hi (ACT) + fp8 lo residual (DVE subtract)
                a0 = b * BAND
                pv = bank[:, :BAND * R].rearrange('p (a r) -> p a r', a=BAND)
                lo_v, hi_v = pad_views(dpad8, a0, BAND)
                nc.scalar.activation(out=hi_v, in_=pv[:], func=COPY)
                nc.vector.tensor_tensor(out=lo_v, in0=pv[:], in1=hi_v,
                                        op=mybir.AluOpType.subtract)

            # conv psum layout: per band one [128, 1024] tile = 2 zero
            # regions; sub-band s2 (2 angles) accumulates over the FLATTENED
            # padded rows at cols [s2*512, s2*512+202) (junk at row-boundary
            # cols, skipped by the epilogue's PADW-strided reads).
            LSUB = PADW + R    # 202

            def emit14(ps, s2, pad_t, w_t, a0, lsub=None):
                """3-product fp8 conv: 14 DoubleRow passes accumulating
                w_hi*x_hi + w_hi*x_lo + w_lo*x_hi over 9 taps."""
                def wc(g, t):
                    return 128 + g * 1152 + t * 128

                def xc(h, t):
                    return (h * HOFF + (t // 3) * PADW + (t % 3)
                            + a0 * PADW)

                pairs = []
                for t in (0, 2, 4, 6):
                    pairs.append(((wc(0, t), xc(1, t)),
                                  (wc(0, t + 1), xc(1, t + 1))))
                for t in (0, 2, 4, 6):
                    pairs.append(((wc(0, t), xc(0, t)),
                                  (wc(0, t + 1), xc(0, t + 1))))
                for t in (0, 2, 4, 6):
                    pairs.append(((wc(1, t), xc(1, t)),
                                  (wc(1, t + 1), xc(1, t + 1))))
                pairs.append(((wc(0, 8), xc(0, 8)), (wc(1, 8), xc(1, 8))))
                # odd 27th product pairs with the zero-weight block (w col
                # 0); the dummy rhs member re-reads tap-0's valid window
                pairs.append(((0, xc(1, 0)), (wc(0, 8), xc(1, 8))))
                wa = w_t[:]
                xa = pad_t[:]
                if lsub is None:
                    lsub = LSUB
                out_v = ps[:, s2 * 512:s2 * 512 + lsub]
                for i, ((w1c, x1c), (w2c, x2c)) in enumerate(pairs):
                    lhsT = AP(wa.tensor, wa.offset + w1c,
                              [[wa.ap[0][0], 128], [w2c - w1c, 2], [1, 128]])
                    rhs = AP(xa.tensor, xa.offset + x1c,
                             [[xa.ap[0][0], 128], [x2c - x1c, 2], [1, lsub]])
                    nc.tensor.matmul(out=out_v, lhsT=lhsT, rhs=rhs,
                                     start=(i == 0),
                                     stop=(i == len(pairs) - 1),
                                     perf_mode=DR)

            def conv_psum_view(ps, ar, sub=0):
                """[p, (sub, al), R] view of the padded conv psum."""
                pa = ps[:]
                if ar == BAND:
                    return AP(pa.tensor, pa.offset,
                              [[pa.ap[0][0], 128], [512, 2], [PADW, 2],
                               [1, R]])
                if ar == 1:
                    return AP(pa.tensor, pa.offset + sub * 512,
                              [[pa.ap[0][0], 128], [1, R]])
                return AP(pa.tensor, pa.offset,
                          [[pa.ap[0][0], 128], [PADW, ar], [1, R]])

            def conv2_band(c):
                a0 = c * BAND
                ps = psc.tile([128, 1024], f32, tag='conv')
                for s2 in range(2):
                    emit14(ps, s2, dpad8, w2_t, a0 + s2 * 2)
                pv = conv_psum_view(ps, BAND)
                hv2 = h2_pad[:].rearrange('c (a r) -> c a r', a=PADW)
                h2v = hv2[:, a0 + 1:a0 + 1 + BAND, 1:1 + R]
                h2v4 = h2v.rearrange('c (x y) r -> c x y r', x=2)
                nc.scalar.activation(out=h2v4, in_=pv, func=RELU,
                                     bias=b2_t[:, :1], scale=SC2)
                lo_v, hi_v = pad_views(hpad8, a0, BAND)
                nc.scalar.activation(out=hi_v, in_=h2v, func=COPY)
                nc.vector.tensor_tensor(out=lo_v, in0=h2v, in1=hi_v,
                                        op=mybir.AluOpType.subtract)

            def conv3_band(c, ar=BAND, s0=0, dve_epi=False, ps=None, sub=0):
                a0 = c * BAND + s0
                if ps is None:
                    ps = psc.tile([128, 1024], f32, tag='conv')
                if ar == 1:
                    emit14(ps, sub, hpad8, w3_t, a0, lsub=R)
                else:
                    for s2 in range(ar // 2):
                        emit14(ps, s2, hpad8, w3_t, a0 + s2 * 2)
                pv = conv_psum_view(ps, ar, sub)
                ob = outb_pool.tile([128, ar * R], f32, tag=f'outb{ar}')
                if ar == BAND:
                    ov = ob[:].rearrange('p (x y r) -> p x y r', x=2, y=2)
                elif ar == 1:
                    ov = ob[:]
                else:
                    ov = ob[:].rearrange('p (a r) -> p a r', a=ar)
                if dve_epi:
                    nc.vector.tensor_scalar(
                        out=ov, in0=pv, scalar1=b3_t[:, :1],
                        scalar2=0.0, op0=mybir.AluOpType.add,
                        op1=mybir.AluOpType.max)
                else:
                    nc.scalar.activation(out=ov, in_=pv, func=RELU,
                                         bias=b3_t[:, :1], scale=1.0)
                nc.sync.dma_start(out=out_d.ap()[:, a0 * R:(a0 + ar) * R],
                                  in_=ob[:])

            banks = {0: psd.tile([128, 512], f32, tag='band',
                                 name='bank_0')}
            zero_bank(0, banks[0])
            for b in range(NBAND):
                # stream the upcoming one-hot bands behind the PE
                if b + 3 < NBAND:
                    issue_oh(b + 3, (nc.sync, nc.gpsimd, nc.scalar)[b % 3])
                if b + 1 < NBAND:
                    banks[b + 1] = psd.tile([128, 512], f32, tag='band',
                                            name=f'bank_{b + 1}')
                    zero_bank(b + 1, banks[b + 1])
                dht_band(b, banks[b])
                drain_band(b, banks[b])
                del banks[b]
                del oh_tiles[b]
                if b >= 2:
                    conv2_band(b - 2)
                if b >= 4:
                    conv3_band(b - 4)
            for c in (NBAND - 2, NBAND - 1):
                conv2_band(c)
            for c in range(NBAND - 4, NBAND - 1):
                conv3_band(c)
            # split the last band so its epilogue/DMA pipeline with the
            # later pieces' matmuls instead of trailing the whole kernel
            conv3_band(NBAND - 1, ar=2, s0=0)
            ps_tail = psc.tile([128, 1024], f32, tag='conv')
            conv3_band(NBAND - 1, ar=1, s0=2, ps=ps_tail, sub=0)
            conv3_band(NBAND - 1, ar=1, s0=3, dve_epi=True, ps=ps_tail,
                       sub=1)

    if split_waits:
        _split_multi_waits(nc, mybir)
    _PROGRAM_CACHE[key] = nc
    return nc


# ----------------------------------------------------------------------------
# entry point
# ----------------------------------------------------------------------------

def make_in_maps(inputs):
    plan = _dht_plan()
    x = np.asarray(inputs['x'], np.float32)
    w1p8, bias18, w2p8, bias2, w3p8, bias3 = _prep_weights(
        *[np.asarray(inputs[k], np.float32) for k in
          ('w1', 'b1', 'g1', 'be1', 'm1', 'v1',
           'w2', 'b2', 'g2', 'be2', 'm2', 'v2',
           'w3', 'b3', 'g3', 'be3', 'm3', 'v3')])
    common = dict(w1p8=w1p8, bias18=bias18, w2p8=w2p8, bias2=bias2,
                  w3p8=w3p8, bias3=bias3, ohtab=plan['ohtab'])
    # x host-permuted: [ci%128, chunk*228 + (ci//128)*128 + p] with
    # p = dy*10+dx, chunk = gy*10+gx; 28-byte zero gaps per chunk
    xb = (x.reshape(N, 2, 128, YBLK, BH, XBLK, BW)
          .transpose(0, 2, 3, 5, 1, 4, 6)      # n, ci128, gy, gx, half, dy, dx
          .reshape(N, 128, NBLK, 2, BP))
    xp = np.zeros((N, 128, NBLK, 228), FP8)
    xp[:, :, :, 0:100] = xb[:, :, :, 0].astype(FP8)
    xp[:, :, :, 128:228] = xb[:, :, :, 1].astype(FP8)
    xp = xp.reshape(N, 128, NBLK * 228)
    return [
        {'x': np.ascontiguousarray(xp[n]), **common}
        for n in range(N)
    ]


def run(inputs, trace=False):
    from concourse.bass_utils import run_bass_kernel_spmd

    nc = _build_program()
    in_maps = make_in_maps(inputs)
    res = run_bass_kernel_spmd(nc, in_maps, core_ids=list(range(N)),
                               trace=trace)
    out = np.stack([res.results[n]['out'].reshape(CMID, H, W)
                    for n in range(N)], axis=0)
    return out.astype(np.float32), res


def kernel(**inputs):
    out, _ = run(inputs, trace=False)
    return out
